# revision 1
# baseline (speedup 1.0000x reference)
"""AttentionalCopula Trainium2 kernel.

Data-parallel over batch: 8 NeuronCores, 2 batch elements per core.
All activations transposed-friendly layouts prepared on host; device does
matmuls in float32r (full PE rate, ~tf32 precision), fp32 vector ops.

Self-contained: hardcodes shapes from the problem spec.
"""
import math
import sys

import numpy as np

sys.path.insert(0, "/opt/trn_rl_repo")

import concourse.bass as bass  # noqa: E402
import concourse.bacc as bacc  # noqa: E402
import concourse.tile as tile  # noqa: E402
import concourse.mybir as mybir  # noqa: E402
from contextlib import ExitStack  # noqa: E402

F32 = mybir.dt.float32
F32R = mybir.dt.float32r
AF = mybir.ActivationFunctionType
ALU = mybir.AluOpType

B, D, NH, NS, NT = 16, 256, 512, 8, 32
NV = NS * NT
L, H, A = 4, 8, 64
HA = H * A
M = 512
R = 128
W = NH + NV
EPS = 1e-5
SCALE = A ** -0.5
NCORES = 8
EPC = B // NCORES  # elems per core

_BUILD_CACHE = {}


def ts(i, n):
    return slice(i * n, (i + 1) * n)


_DEBUG = False
_NPHASE = 99  # debug bisect: 1=ds only, 2=+keys/vals, 3=+attention, 4=+ln/ff, 99=full


def _build(use_ff_bias, use_de_bias, ln_affine):
    nc = bacc.Bacc(None, target_bir_lowering=False)

    def P(name, shape, out=False, dt=F32):
        return nc.declare_dram_parameter(name, shape, dt, isOutput=out)

    kiT_d = P("kiT", (EPC, 258, W), dt=F32R)
    kw_d = P("kwp", (L, 258, HA), dt=F32R)
    vw_d = P("vwp", (L, 258, HA), dt=F32R)
    ds_d = P("dswp", (258, HA), dt=F32R)
    f1_d = P("ffw1", (L, 513, M), dt=F32R)
    f2_d = P("ffw2", (L, 513, M), dt=F32R)
    f3_d = P("ffw3", (L, 513, HA), dt=F32R)
    dew_d = P("dew", (HA, R), dt=F32R)
    deb_d = P("deb", (1, R), dt=F32R)
    mask_d = P("maskmul", (128, 128))
    oh_d = P("onehot", (EPC, 2, 128, R))
    id_d = P("ident", (128, 128), dt=F32R)
    wv_d = P("wv0", (128, 1))
    onesr_d = P("onesrow", (1, W), dt=F32R)
    onesc_d = P("onescol", (128, 1), dt=F32R)
    vones_d = P("vones", (128, 96), dt=F32R)
    if ln_affine:
        lnp_d = P("lnp", (L, 4, HA))
    out_d = P("out", (1, EPC), out=True)
    if _DEBUG:
        dbg_keys_d = P("dbg_keys", (128, 4, W), out=True)
        dbg_vals_d = P("dbg_vals", (128, 6, 8, 66), out=True)
        dbg_exp_d = P("dbg_exp", (128, 1536), out=True)
        dbg_att0_d = P("dbg_att0", (128, 2, HA), out=True)
        dbg_attr_d = P("dbg_attr", (128, 2, HA), out=True)
        dbg_att1_d = P("dbg_att1", (128, 2, HA), out=True)
        dbg_lg_d = P("dbg_lg", (2, 128, R), out=True)

    with tile.TileContext(nc) as tc, ExitStack() as ctx:
        const = ctx.enter_context(tc.tile_pool(name="const", bufs=1))
        kpool = ctx.enter_context(tc.tile_pool(name="kvw", bufs=2))
        fpool = ctx.enter_context(tc.tile_pool(name="ffw", bufs=2))
        iopool = ctx.enter_context(tc.tile_pool(name="io", bufs=2))
        kvpool = ctx.enter_context(tc.tile_pool(name="keys", bufs=1))
        epool = ctx.enter_context(tc.tile_pool(name="exp", bufs=3))
        apool = ctx.enter_context(tc.tile_pool(name="att", bufs=5))
        tpool = ctx.enter_context(tc.tile_pool(name="attT", bufs=3))
        ftpool = ctx.enter_context(tc.tile_pool(name="ffT", bufs=2))
        spool = ctx.enter_context(tc.tile_pool(name="small", bufs=4))
        ps_s = ctx.enter_context(tc.tile_pool(name="ps_s", bufs=1, space="PSUM"))
        ps_b = ctx.enter_context(tc.tile_pool(name="ps_b", bufs=4, space="PSUM"))
        ps_a = ctx.enter_context(tc.tile_pool(name="ps_a", bufs=1, space="PSUM"))

        dma = nc.sync.dma_start

        # ---- constants ----
        ident = const.tile([128, 128], F32R, tag="ident")
        dma(ident[:], id_d.ap())
        maskm = const.tile([128, 128], F32, tag="maskm")
        dma(maskm[:], mask_d.ap())
        onehot_t = const.tile([128, EPC * 2, R], F32, tag="onehot")
        for e in range(EPC):
            for vt in range(2):
                dma(onehot_t[:, e * 2 + vt, :], oh_d.ap()[e, vt])
        wv0 = const.tile([128, 1], F32, tag="wv0")
        dma(wv0[:], wv_d.ap())
        ones_row = const.tile([1, W], F32R, tag="ones_row")
        dma(ones_row[:], onesr_d.ap())
        ones_col = const.tile([128, 1], F32R, tag="ones_col")
        dma(ones_col[:], onesc_d.ap())
        dsw_t = const.tile([128, 2, HA], F32R, tag="dsw")
        dma(dsw_t[:], ds_d.ap()[0:256].rearrange("(a p) n -> p a n", p=128))
        dsu_t = const.tile([2, HA], F32R, tag="dsu")
        dma(dsu_t[:], ds_d.ap()[256:258])
        dew_t = const.tile([128, 4, R], F32R, tag="dew")
        dma(dew_t[:], dew_d.ap().rearrange("(a p) n -> p a n", p=128))
        deb_t = const.tile([1, R], F32R, tag="deb")
        dma(deb_t[:], deb_d.ap())
        if use_ff_bias:
            ffb_t = const.tile([12, M], F32R, tag="ffb")
            for mi, fd in enumerate((f1_d, f2_d, f3_d)):
                for l in range(L):
                    dma(ffb_t[mi * 4 + l: mi * 4 + l + 1, :], fd.ap()[l, 512:513, :])
        if ln_affine:
            lnp_t = const.tile([16, HA], F32, tag="lnp")
            for l in range(L):
                for j in range(4):
                    dma(lnp_t[l * 4 + j: l * 4 + j + 1, :], lnp_d.ap()[l, j: j + 1, :])
        res_sb = const.tile([1, EPC], F32, tag="res")
        if _NPHASE < 99:
            nc.gpsimd.memset(res_sb[:], 0.0)
        keysT = const.tile([128, 4, W], F32R, tag="keys")
        vals = const.tile([128, 6, 8, 66], F32R, tag="vals")
        dma(vals[:, :, :, 64:66], vones_d.ap().rearrange("p (a b c) -> p a b c", a=6, b=8))
        eps_t = const.tile([128, 1], F32, tag="eps")
        nc.gpsimd.memset(eps_t[:], EPS)
        sc8_t = const.tile([128, 1], F32, tag="sc8")
        nc.gpsimd.memset(sc8_t[:], SCALE)
        neg1_t = const.tile([1, 1], F32, tag="neg1")
        nc.gpsimd.memset(neg1_t[:], -1.0)
        fbias_t = const.tile([1, 1], F32, tag="fbias")
        nc.gpsimd.memset(fbias_t[:], -(NV - 1) * math.log(R))

        evac_ctr = [0]

        def evac(out_ap, in_ap):
            # PSUM->SBUF copies: 2/3 on DVE, 1/3 on ACT (ACT is exp-bound)
            if evac_ctr[0] % 3 < 2:
                nc.vector.tensor_copy(out_ap, in_ap)
            else:
                nc.scalar.copy(out_ap, in_ap)
            evac_ctr[0] += 1

        def mm(ps_ap, chunks, dt=F32R):
            n = len(chunks)
            for i, (lh, rh) in enumerate(chunks):
                nc.tensor.matmul(ps_ap, lh, rh,
                                 start=(i == 0), stop=(i == n - 1))

        def ln_apply(out_ap, in_ap, l, which, vt, small):
            """LayerNorm along free dim (HA) of [128, HA] tile."""
            st6 = small.tile([128, 6], F32, tag="st6")
            nc.vector.bn_stats(st6[:], in_ap)
            mv = small.tile([128, 2], F32, tag="mv")
            nc.vector.bn_aggr(mv[:], st6[:])
            sd = small.tile([128, 1], F32, tag="sd")
            nc.scalar.activation(sd[:], mv[:, 1:2], AF.Sqrt, bias=eps_t[:, 0:1])
            rs = small.tile([128, 1], F32, tag="rs")
            nc.vector.reciprocal(rs[:], sd[:])
            nb = small.tile([128, 1], F32, tag="nb")
            nc.vector.tensor_scalar(nb[:], mv[:, 0:1], rs[:, 0:1], -1.0,
                                    op0=ALU.mult, op1=ALU.mult)
            if not ln_affine:
                nc.scalar.activation(out_ap, in_ap, AF.Identity,
                                     bias=nb[:, 0:1], scale=rs[:, 0:1])
            else:
                t0 = small.tile([128, HA], F32, tag="lnt0")
                nc.scalar.activation(t0[:], in_ap, AF.Identity,
                                     bias=nb[:, 0:1], scale=rs[:, 0:1])
                gb = small.tile([128, HA], F32, tag="lngb")
                gi = l * 4 + (0 if which == 1 else 2)
                nc.gpsimd.partition_broadcast(gb[:], lnp_t[gi: gi + 1, :])
                nc.vector.tensor_mul(t0[:], t0[:], gb[:])
                bi = gi + 1
                nc.gpsimd.partition_broadcast(gb[:], lnp_t[bi: bi + 1, :])
                nc.vector.tensor_add(out_ap, t0[:], gb[:])

        # ================== per batch element ==================
        for e in range(EPC):
            ki0 = iopool.tile([128, W], F32R, tag="ki0")
            ki1 = iopool.tile([128, W], F32R, tag="ki1")
            kiu = iopool.tile([2, W], F32R, tag="kiu")
            dma(ki0[:], kiT_d.ap()[e, 0:128])
            dma(ki1[:], kiT_d.ap()[e, 128:256])
            dma(kiu[:], kiT_d.ap()[e, 256:258])
            kich = [ki0, ki1]

            # ---- initial att (natural [v,ha]) and attT ([ha,v]) ----
            att = apool.tile([128, 2, HA], F32R, tag="att")
            for vt in range(2):
                ps = ps_b.tile([128, 512], F32, tag="psb")
                mm(ps[:], [(ki0[:, 512 + vt * 128: 512 + (vt + 1) * 128], dsw_t[:, 0, :]),
                           (ki1[:, 512 + vt * 128: 512 + (vt + 1) * 128], dsw_t[:, 1, :]),
                           (kiu[:, 512 + vt * 128: 512 + (vt + 1) * 128], dsu_t[:, :])])
                evac(att[:, vt, :], ps[:])
            attT = tpool.tile([128, 4, NV], F32R, tag="attT")
            for t in range(4):
                ps = ps_b.tile([128, 512], F32, tag="psb")
                mm(ps[:, 0:NV], [(dsw_t[:, 0, ts(t, 128)], ki0[:, 512:768]),
                                 (dsw_t[:, 1, ts(t, 128)], ki1[:, 512:768]),
                                 (dsu_t[:, ts(t, 128)], kiu[:, 512:768])])
                evac(attT[:, t, :], ps[:, 0:NV])

            # ================== layers ==================
            for l in range(L if _NPHASE in (5, 99) else (1 if _NPHASE >= 2 else 0)):
                kw_t = kpool.tile([128, 2, HA], F32R, tag="kw")
                dma(kw_t[:], kw_d.ap()[l, 0:256].rearrange("(a p) n -> p a n", p=128))
                vw_t = kpool.tile([128, 2, HA], F32R, tag="vw")
                dma(vw_t[:], vw_d.ap()[l, 0:256].rearrange("(a p) n -> p a n", p=128))
                kvu_t = kpool.tile([2, 2, HA], F32R, tag="kvu")
                dma(kvu_t[:, 0, :], kw_d.ap()[l, 256:258])
                dma(kvu_t[:, 1, :], vw_d.ap()[l, 256:258])
                ffw1_t = fpool.tile([128, 4, M], F32R, tag="f1")
                dma(ffw1_t[:], f1_d.ap()[l, 0:512].rearrange("(a p) n -> p a n", p=128))
                ffw2_t = fpool.tile([128, 4, M], F32R, tag="f2")
                dma(ffw2_t[:], f2_d.ap()[l, 0:512].rearrange("(a p) n -> p a n", p=128))
                ffw3_t = fpool.tile([128, 4, HA], F32R, tag="f3")
                dma(ffw3_t[:], f3_d.ap()[l, 0:512].rearrange("(a p) n -> p a n", p=128))

                # ---- keysT [ha, w] ----
                for t in range(4):
                    for (wlo, wn) in ((0, 512), (512, 256)):
                        ps = ps_b.tile([128, 512], F32, tag="psb")
                        mm(ps[:, 0:wn],
                           [(kw_t[:, 0, ts(t, 128)], ki0[:, wlo:wlo + wn]),
                            (kw_t[:, 1, ts(t, 128)], ki1[:, wlo:wlo + wn]),
                            (kvu_t[:, 0, ts(t, 128)], kiu[:, wlo:wlo + wn])])
                        evac(keysT[:, t, wlo:wlo + wn], ps[:, 0:wn])

                # ---- vals [w, (h,a)] with ones column per head ----
                for wt in range(6):
                    ps = ps_b.tile([128, 512], F32, tag="psb")
                    mm(ps[:], [(ki0[:, ts(wt, 128)], vw_t[:, 0, :]),
                               (ki1[:, ts(wt, 128)], vw_t[:, 1, :]),
                               (kiu[:, ts(wt, 128)], kvu_t[:, 1, :])])
                    evac(vals[:, wt, :, 0:64], ps[:].rearrange("p (h a) -> p h a", h=8))

                # ---- attention ----
                att_res = apool.tile([128, 2, HA], F32R, tag="att")
                for h in range(H if _NPHASE >= 3 else 0):
                    t, base = h // 2, (h % 2) * 64
                    ps_st = ps_s.tile([128, 1536], F32, tag="s")
                    for wt in range(6):
                        nc.tensor.matmul(
                            ps_st[:, ts(wt, 256)],
                            keysT[base:base + 64, t, ts(wt, 128)],
                            attT[base:base + 64, t, :],
                            start=True, stop=True)
                    expT = epool.tile([128, 1536], F32R, tag="exp")
                    nc.scalar.activation(expT[:], ps_st[:], AF.Exp, scale=sc8_t[:, 0:1])
                    nc.vector.tensor_mul(expT[:, 1024:1152], expT[:, 1024:1152], maskm[:])
                    nc.vector.tensor_mul(expT[:, 1408:1536], expT[:, 1408:1536], maskm[:])
                    nc.vector.tensor_scalar_mul(expT[:, 1280:1408],
                                                expT[:, 1280:1408], 0.0)
                    ps_at = ps_a.tile([66, 256], F32, tag="a")
                    for wt in range(6):
                        nc.tensor.matmul(ps_at[:], vals[:, wt, h, :],
                                         expT[:, ts(wt, 256)],
                                         start=(wt == 0), stop=(wt == 5))
                    aT_s = spool.tile([66, 256], F32R, tag="aTs")
                    evac(aT_s[:], ps_at[:])
                    if _DEBUG and e == 0 and l == 0 and h == 0:
                        dma(dbg_exp_d.ap()[:], expT[:].bitcast(F32))
                    ps_tr = ps_b.tile([128, 512], F32R, tag="psb")
                    rec = spool.tile([128, 2], F32, tag="rec")
                    for half in range(2):
                        nc.tensor.transpose(ps_tr[:, half * 66:half * 66 + 66],
                                            aT_s[:, ts(half, 128)], ident[0:66, 0:66])
                    for half in range(2):
                        nc.vector.reciprocal(rec[:, half:half + 1],
                                             ps_tr[:, half * 66 + 64:half * 66 + 65])
                    for half in range(2):
                        nc.vector.scalar_tensor_tensor(
                            att_res[:, half, ts(h, 64)],
                            ps_tr[:, half * 66:half * 66 + 64],
                            rec[:, half:half + 1],
                            att[:, half, ts(h, 64)],
                            op0=ALU.mult, op1=ALU.add)

                if _DEBUG and e == 0 and l == 0:
                    dma(dbg_keys_d.ap()[:], keysT[:].bitcast(F32))
                    dma(dbg_vals_d.ap()[:], vals[:].bitcast(F32))
                    dma(dbg_att0_d.ap()[:], att[:].bitcast(F32))
                    dma(dbg_attr_d.ap()[:], att_res[:].bitcast(F32))
                # ---- LN1 ----
                if _NPHASE < 4:
                    continue
                att1 = apool.tile([128, 2, HA], F32R, tag="att")
                for vt in range(2):
                    ln_apply(att1[:, vt, :], att_res[:, vt, :], l, 1, vt, spool)
                if _DEBUG and e == 0 and l == 0:
                    dma(dbg_att1_d.ap()[:], att1[:].bitcast(F32))
                att1T = tpool.tile([128, 4, NV], F32R, tag="attT")
                for c in range(4):
                    ps_tr = ps_b.tile([128, 512], F32R, tag="psb")
                    for vt in range(2):
                        nc.tensor.transpose(ps_tr[:, ts(vt, 128)],
                                            att1[:, vt, ts(c, 128)], ident[:])
                    evac(att1T[:, c, :], ps_tr[:, 0:256])

                # ---- FF ----
                ff1T = ftpool.tile([128, 4, NV], F32R, tag="ffT")
                for mt in range(4):
                    ps = ps_b.tile([128, 512], F32, tag="psb")
                    ch = [(ffw1_t[:, c, ts(mt, 128)], att1T[:, c, :]) for c in range(4)]
                    if use_ff_bias:
                        ch.append((ffb_t[l:l + 1, ts(mt, 128)], ones_row[:, 0:NV]))
                    mm(ps[:, 0:NV], ch)
                    nc.vector.tensor_scalar_max(ff1T[:, mt, :], ps[:, 0:NV], 0.0)
                ff2T = ftpool.tile([128, 4, NV], F32R, tag="ffT")
                for mt in range(4):
                    ps = ps_b.tile([128, 512], F32, tag="psb")
                    ch = [(ffw2_t[:, c, ts(mt, 128)], ff1T[:, c, :]) for c in range(4)]
                    if use_ff_bias:
                        ch.append((ffb_t[4 + l:5 + l, ts(mt, 128)], ones_row[:, 0:NV]))
                    mm(ps[:, 0:NV], ch)
                    nc.vector.tensor_scalar_max(ff2T[:, mt, :], ps[:, 0:NV], 0.0)
                att2_res = apool.tile([128, 2, HA], F32R, tag="att")
                for c in range(4):
                    ps3 = ps_b.tile([128, 512], F32, tag="psb")
                    ch = [(ffw3_t[:, k, ts(c, 128)], ff2T[:, k, :]) for k in range(4)]
                    if use_ff_bias:
                        ch.append((ffb_t[8 + l:9 + l, ts(c, 128)], ones_row[:, 0:NV]))
                    mm(ps3[:, 0:NV], ch)
                    f3s = ftpool.tile([128, NV], F32R, tag="f3s")
                    evac(f3s[:], ps3[:, 0:NV])
                    ps_tr = ps_b.tile([128, 512], F32R, tag="psb")
                    for vt in range(2):
                        nc.tensor.transpose(ps_tr[:, ts(vt, 128)],
                                            f3s[:, ts(vt, 128)], ident[:])
                    for vt in range(2):
                        nc.vector.tensor_add(att2_res[:, vt, ts(c, 128)],
                                             ps_tr[:, ts(vt, 128)],
                                             att1[:, vt, ts(c, 128)])

                # ---- LN2 ----
                att2 = apool.tile([128, 2, HA], F32R, tag="att")
                for vt in range(2):
                    ln_apply(att2[:, vt, :], att2_res[:, vt, :], l, 2, vt, spool)
                att2T = tpool.tile([128, 4, NV], F32R, tag="attT")
                for c in range(4):
                    ps_tr = ps_b.tile([128, 512], F32R, tag="psb")
                    for vt in range(2):
                        nc.tensor.transpose(ps_tr[:, ts(vt, 128)],
                                            att2[:, vt, ts(c, 128)], ident[:])
                    evac(att2T[:, c, :], ps_tr[:, 0:256])
                att, attT = att2, att2T

            # ================== loss head ==================
            if _NPHASE in (5,) or _NPHASE < 4:
                continue
            if _NPHASE < 99 and _NPHASE >= 10:
                pass
            q = spool.tile([128, 2], F32R, tag="q")
            for vt in range(2):
                ps = ps_b.tile([128, 512], F32, tag="psb")
                ch = [(attT[:, c, ts(vt, 128)], dew_t[:, c, :]) for c in range(4)]
                if use_de_bias:
                    ch.append((ones_row[0:1, ts(vt, 128)], deb_t[:, :]))
                mm(ps[:, 0:R], ch)
                lg = spool.tile([128, R], F32, tag="lg")
                evac(lg[:], ps[:, 0:R])
                if _DEBUG and e == 0:
                    dma(dbg_lg_d.ap()[vt], lg[:])
                if _NPHASE < 11:
                    continue
                scr = spool.tile([128, R], F32, tag="scr")
                se = spool.tile([128, 1], F32, tag="se")
                nc.scalar.activation(scr[:], lg[:], AF.Exp, accum_out=se[:])
                if _NPHASE < 12:
                    continue
                lse = spool.tile([128, 1], F32, tag="lse")
                nc.scalar.activation(lse[:], se[:], AF.Ln)
                if _NPHASE < 13:
                    continue
                pick = spool.tile([128, 1], F32, tag="pick")
                nc.vector.tensor_mul(scr[:], lg[:], onehot_t[:, e * 2 + vt, :])
                nc.vector.tensor_reduce(pick[:], scr[:], mybir.AxisListType.X,
                                        ALU.add)
                if _NPHASE < 14:
                    continue
                nc.vector.scalar_tensor_tensor(
                    q[:, vt:vt + 1], lse[:], -1.0, pick[:],
                    op0=ALU.mult, op1=ALU.add)
            if _NPHASE < 15:
                continue
            nc.vector.tensor_mul(q[:, 0:1], q[:, 0:1], wv0[:])
            ps_l = ps_a.tile([66, 256], F32, tag="a")
            nc.tensor.matmul(ps_l[0:1, 0:2], ones_col[:], q[:, 0:2],
                             start=True, stop=True)
            tot = spool.tile([1, 1], F32, tag="tot")
            nc.vector.tensor_reduce(tot[:], ps_l[0:1, 0:2], mybir.AxisListType.X,
                                    ALU.add)
            nc.scalar.activation(res_sb[0:1, e:e + 1], tot[0:1, 0:1], AF.Identity,
                                 scale=neg1_t[0:1, 0:1], bias=fbias_t[0:1, 0:1])
        dma(out_d.ap()[0:1, :], res_sb[:])

    nc.finalize()
    return nc


def _prep_inputs(inputs):
    hist_encoded = np.asarray(inputs["hist_encoded"], np.float32)
    hist_true_u = np.asarray(inputs["hist_true_u"], np.float32)
    pred_encoded = np.asarray(inputs["pred_encoded"], np.float32)
    pred_true_u = np.asarray(inputs["pred_true_u"], np.float32)
    key_w = np.asarray(inputs["key_w"], np.float32)
    key_b = np.asarray(inputs["key_b"], np.float32)
    val_w = np.asarray(inputs["val_w"], np.float32)
    val_b = np.asarray(inputs["val_b"], np.float32)
    ds_w = np.asarray(inputs["ds_w"], np.float32)
    ds_b = np.asarray(inputs["ds_b"], np.float32)
    ff_w1 = np.asarray(inputs["ff_w1"], np.float32)
    ff_b1 = np.asarray(inputs["ff_b1"], np.float32)
    ff_w2 = np.asarray(inputs["ff_w2"], np.float32)
    ff_b2 = np.asarray(inputs["ff_b2"], np.float32)
    ff_w3 = np.asarray(inputs["ff_w3"], np.float32)
    ff_b3 = np.asarray(inputs["ff_b3"], np.float32)
    de_w = np.asarray(inputs["de_w"], np.float32)
    de_b = np.asarray(inputs["de_b"], np.float32)
    ln1_g = np.asarray(inputs["ln1_g"], np.float32)
    ln1_b = np.asarray(inputs["ln1_b"], np.float32)
    ln2_g = np.asarray(inputs["ln2_g"], np.float32)
    ln2_b = np.asarray(inputs["ln2_b"], np.float32)

    # kiT per batch elem: [258, W]
    enc = np.concatenate([hist_encoded, pred_encoded], axis=1)  # [B, W, D]
    u = np.concatenate([hist_true_u, pred_true_u], axis=1)      # [B, W]
    kiT = np.empty((B, 258, W), np.float32)
    kiT[:, 0:256, :] = enc.transpose(0, 2, 1)
    kiT[:, 256, :] = u
    kiT[:, 257, :] = 1.0

    def pack_kv(wt, bt):  # [L,H,257,A],[L,H,A] -> [L,258,HA]
        p = np.empty((L, 258, HA), np.float32)
        p[:, 0:257, :] = wt.transpose(0, 2, 1, 3).reshape(L, 257, HA)
        p[:, 257, :] = bt.reshape(L, HA)
        return p

    kwp = pack_kv(key_w, key_b)
    vwp = pack_kv(val_w, val_b)

    dswp = np.zeros((258, HA), np.float32)
    dswp[0:256] = ds_w
    dswp[257] = ds_b

    def pack_ff(wt, bt, n):
        p = np.empty((L, 513, n), np.float32)
        p[:, 0:512, :] = wt
        p[:, 512, :] = bt
        return p

    ffw1 = pack_ff(ff_w1, ff_b1, M)
    ffw2 = pack_ff(ff_w2, ff_b2, M)
    ffw3 = pack_ff(ff_w3, ff_b3, HA)

    rho = np.arange(128)[:, None]
    vv = np.arange(128)[None, :]
    maskmul = (vv > rho).astype(np.float32)  # 0 where v <= rho (masked)

    tgt = np.clip(np.floor(pred_true_u * R).astype(np.int64), 0, R - 1)  # [B, NV]
    onehot = np.zeros((B, 2, 128, R), np.float32)
    for vt in range(2):
        idx = tgt[:, vt * 128:(vt + 1) * 128]
        onehot[np.arange(B)[:, None], vt, np.arange(128)[None, :], idx] = 1.0
    onehot[:, 0, 0, :] = 0.0  # exclude v=0

    ident = np.eye(128, dtype=np.float32)
    wv0 = np.ones((128, 1), np.float32)
    wv0[0, 0] = 0.0

    use_ff_bias = bool(np.any(ff_b1) or np.any(ff_b2) or np.any(ff_b3))
    use_de_bias = bool(np.any(de_b))
    ln_affine = bool(np.any(ln1_g != 1.0) or np.any(ln1_b) or
                     np.any(ln2_g != 1.0) or np.any(ln2_b))
    lnp = np.stack([ln1_g, ln1_b, ln2_g, ln2_b], axis=1)  # [L,4,HA]

    shared = {
        "kwp": kwp, "vwp": vwp, "dswp": dswp,
        "ffw1": ffw1, "ffw2": ffw2, "ffw3": ffw3,
        "dew": de_w, "deb": de_b.reshape(1, R),
        "maskmul": maskmul, "ident": ident, "wv0": wv0,
        "onesrow": np.ones((1, W), np.float32),
        "onescol": np.ones((128, 1), np.float32),
        "vones": np.tile(np.array([1.0, 0.0], np.float32), 48).reshape(1, 96).repeat(128, 0),
    }
    if ln_affine:
        shared["lnp"] = lnp
    in_maps = []
    for c in range(NCORES):
        m = dict(shared)
        m["kiT"] = kiT[c * EPC:(c + 1) * EPC]
        m["onehot"] = onehot[c * EPC:(c + 1) * EPC]
        in_maps.append(m)
    return in_maps, (use_ff_bias, use_de_bias, ln_affine)


def _get_nc(flags):
    if flags not in _BUILD_CACHE:
        _BUILD_CACHE[flags] = _build(*flags)
    return _BUILD_CACHE[flags]


def _run(inputs, trace=False):
    from concourse.bass_utils import run_bass_kernel_spmd
    in_maps, flags = _prep_inputs(inputs)
    nc = _get_nc(flags)
    res = run_bass_kernel_spmd(nc, in_maps, list(range(NCORES)), trace=trace)
    out = np.concatenate([res.results[c]["out"].reshape(EPC)
                          for c in range(NCORES)])
    return out.astype(np.float32), res


def kernel(**inputs) -> np.ndarray:
    out, _ = _run(inputs, trace=False)
    return out



# revision 8
# speedup vs baseline: 1.2039x; 1.2039x over previous
"""AttentionalCopula Trainium2 kernel.

Data-parallel over batch: 8 NeuronCores, 2 batch elements per core.
Key perf structure vs the naive version:
  - weights DMA'd once per layer (shared by both batch elements)
  - keys/vals double-buffered so layer l+1 projection GEMMs overlap
    layer l attention (keeps TensorE dense -> HAM clock stays at 2.4GHz)
  - QK scores chunked into [128,512] psum tiles so exp (ACT) pipelines
    behind QK (PE) at chunk granularity
  - u-row contribution of key/val projections folded into the psum
    evacuation as a DVE scalar_tensor_tensor (drops the K=2 matmul chains)
  - attention masks + dead-block zeroing on GpSimd (otherwise idle)
  - loss head restructured so all matmuls have moving free dim >= 256
    (fp32r matmuls with N<256 run at 1/4 rate)

Self-contained: hardcodes shapes from the problem spec.
"""
import math
import sys

import numpy as np

sys.path.insert(0, "/opt/trn_rl_repo")

import concourse.bass as bass  # noqa: E402
import concourse.bacc as bacc  # noqa: E402
import concourse.tile as tile  # noqa: E402
import concourse.mybir as mybir  # noqa: E402
from contextlib import ExitStack  # noqa: E402

F32 = mybir.dt.float32
F32R = mybir.dt.float32r
AF = mybir.ActivationFunctionType
ALU = mybir.AluOpType

B, D, NH, NS, NT = 16, 256, 512, 8, 32
NV = NS * NT
L, H, A = 4, 8, 64
HA = H * A
M = 512
R = 128
W = NH + NV
EPS = 1e-5
SCALE = A ** -0.5
NCORES = 8
EPC = B // NCORES  # elems per core

_BUILD_CACHE = {}


def ts(i, n):
    return slice(i * n, (i + 1) * n)


def _build(kv_bias, ds_bias, ln_affine):
    nc = bacc.Bacc(None, target_bir_lowering=False)

    def P(name, shape, out=False, dt=F32):
        return nc.declare_dram_parameter(name, shape, dt, isOutput=out)

    kiT_d = P("kiT", (EPC, 258, W), dt=F32R)
    ucol_d = P("ucol", (EPC, 128, 6))
    kw_d = P("kwp", (L, 258, HA), dt=F32R)
    vw_d = P("vwp", (L, 258, HA), dt=F32R)
    ds_d = P("dswp", (258, HA), dt=F32R)
    f1_d = P("ffw1", (L, 512, M), dt=F32R)
    f2_d = P("ffw2", (L, 512, M), dt=F32R)
    f3_d = P("ffw3", (L, 512, HA), dt=F32R)
    ffb_d = P("ffbcol", (128, 3 * L * 4))
    kwu_d = P("kwucol", (128, L * 4))
    dew_d = P("dew", (HA, R), dt=F32R)
    deb_d = P("debcol", (128, 1))
    mask_d = P("maskmul", (128, 128))
    ohT_d = P("onehotT", (EPC, 128, NV))
    id_d = P("ident", (128, 128), dt=F32R)
    onesc_d = P("onescol", (128, 1), dt=F32R)
    vones_d = P("vones", (128, 96), dt=F32R)
    if kv_bias:
        kbc_d = P("kbcol", (128, 2 * L * 4))
    if ln_affine:
        lnp_d = P("lnp", (L, 4, HA))
    out_d = P("out", (1, EPC), out=True)

    with tile.TileContext(nc) as tc, ExitStack() as ctx:
        const = ctx.enter_context(tc.tile_pool(name="const", bufs=1))
        wpool = ctx.enter_context(tc.tile_pool(name="wts", bufs=2))
        fpool = ctx.enter_context(tc.tile_pool(name="ffw", bufs=1))
        kvpool = ctx.enter_context(tc.tile_pool(name="keys", bufs=2))
        epool = ctx.enter_context(tc.tile_pool(name="exp", bufs=2))
        apool = ctx.enter_context(tc.tile_pool(name="att", bufs=3))
        tpool = ctx.enter_context(tc.tile_pool(name="attT", bufs=2))
        ftpool = ctx.enter_context(tc.tile_pool(name="ffT", bufs=2))
        spool = ctx.enter_context(tc.tile_pool(name="small", bufs=3))
        lpool = ctx.enter_context(tc.tile_pool(name="loss", bufs=2))
        ps_sc = ctx.enter_context(tc.tile_pool(name="ps_sc", bufs=3, space="PSUM"))
        ps_b = ctx.enter_context(tc.tile_pool(name="ps_b", bufs=3, space="PSUM"))
        ps_a = ctx.enter_context(tc.tile_pool(name="ps_a", bufs=2, space="PSUM"))

        dma = nc.sync.dma_start

        # ---- constants ----
        ident = const.tile([128, 128], F32R, tag="ident")
        dma(ident[:], id_d.ap())
        maskm = const.tile([128, 128], F32, tag="maskm")
        dma(maskm[:], mask_d.ap())
        ohT_t = const.tile([128, EPC, NV], F32, tag="ohT")
        for e in range(EPC):
            dma(ohT_t[:, e, :], ohT_d.ap()[e])
        onesc = const.tile([128, 1], F32R, tag="onesc")
        dma(onesc[:], onesc_d.ap())
        vones_t = const.tile([128, 96], F32R, tag="vones")
        dma(vones_t[:], vones_d.ap())
        dsw_t = const.tile([128, 2, HA], F32R, tag="dsw")
        dma(dsw_t[:], ds_d.ap()[0:256].rearrange("(a p) n -> p a n", p=128))
        if ds_bias:
            dsu_t = const.tile([2, HA], F32R, tag="dsu")
            dma(dsu_t[:], ds_d.ap()[256:258])
        dew_t = const.tile([128, 4, R], F32R, tag="dew")
        dma(dew_t[:], dew_d.ap().rearrange("(a p) n -> p a n", p=128))
        debcol = const.tile([128, 1], F32, tag="debcol")
        dma(debcol[:], deb_d.ap())
        ffb_t = const.tile([128, 3 * L * 4], F32, tag="ffb")
        dma(ffb_t[:], ffb_d.ap())
        kwu_t = const.tile([128, L * 4], F32, tag="kwu")
        dma(kwu_t[:], kwu_d.ap())
        if kv_bias:
            kbc_t = const.tile([128, 2 * L * 4], F32, tag="kbc")
            dma(kbc_t[:], kbc_d.ap())
        if ln_affine:
            lnp_t = const.tile([16, HA], F32, tag="lnp")
            for l in range(L):
                for j in range(4):
                    dma(lnp_t[l * 4 + j: l * 4 + j + 1, :], lnp_d.ap()[l, j: j + 1, :])
        res_sb = const.tile([1, EPC], F32, tag="res")
        eps_t = const.tile([128, 1], F32, tag="eps")
        nc.gpsimd.memset(eps_t[:], EPS)
        sc8_t = const.tile([128, 1], F32, tag="sc8")
        nc.gpsimd.memset(sc8_t[:], SCALE)
        neg1_t = const.tile([1, 1], F32, tag="neg1")
        nc.gpsimd.memset(neg1_t[:], -1.0)
        fbias_t = const.tile([1, 1], F32, tag="fbias")
        nc.gpsimd.memset(fbias_t[:], -(NV - 1) * math.log(R))

        evac_ctr = [0]

        def evac(out_ap, in_ap):
            # PSUM->SBUF copies, alternating DVE / ACT
            if evac_ctr[0] % 2 == 0:
                nc.vector.tensor_copy(out_ap, in_ap)
            else:
                nc.scalar.copy(out_ap, in_ap)
            evac_ctr[0] += 1

        def mm(ps_ap, chunks):
            n = len(chunks)
            for i, (lh, rh) in enumerate(chunks):
                nc.tensor.matmul(ps_ap, lh, rh,
                                 start=(i == 0), stop=(i == n - 1))

        def ln_apply(out_ap, in_ap, l, which):
            """LayerNorm along free dim (HA) of a [128, HA] slice."""
            st6 = spool.tile([128, 6], F32, tag="st6")
            nc.vector.bn_stats(st6[:], in_ap)
            mv = spool.tile([128, 2], F32, tag="mv")
            nc.vector.bn_aggr(mv[:], st6[:])
            sd = spool.tile([128, 1], F32, tag="sd")
            nc.scalar.activation(sd[:], mv[:, 1:2], AF.Sqrt, bias=eps_t[:, 0:1])
            rs = spool.tile([128, 1], F32, tag="rs")
            nc.vector.reciprocal(rs[:], sd[:])
            nb = spool.tile([128, 1], F32, tag="nb")
            nc.vector.tensor_scalar(nb[:], mv[:, 0:1], rs[:, 0:1], -1.0,
                                    op0=ALU.mult, op1=ALU.mult)
            if not ln_affine:
                # (x * rs) + nb  on DVE
                nc.vector.tensor_scalar(out_ap, in_ap, rs[:, 0:1], nb[:, 0:1],
                                        op0=ALU.mult, op1=ALU.add)
            else:
                t0 = spool.tile([128, HA], F32, tag="lnt0")
                nc.vector.tensor_scalar(t0[:], in_ap, rs[:, 0:1], nb[:, 0:1],
                                        op0=ALU.mult, op1=ALU.add)
                gb = spool.tile([128, HA], F32, tag="lngb")
                gi = l * 4 + (0 if which == 1 else 2)
                nc.gpsimd.partition_broadcast(gb[:], lnp_t[gi: gi + 1, :])
                nc.vector.tensor_mul(t0[:], t0[:], gb[:])
                bi = gi + 1
                nc.gpsimd.partition_broadcast(gb[:], lnp_t[bi: bi + 1, :])
                nc.vector.tensor_add(out_ap, t0[:], gb[:])

        # ---- per-elem inputs + dimension-shift init ----
        ki = {}
        ubc = {}
        ucol_t = {}
        state = {}
        for e in range(EPC):
            ki0 = const.tile([128, W], F32R, tag=f"ki0_{e}")
            ki1 = const.tile([128, W], F32R, tag=f"ki1_{e}")
            kiu = const.tile([2, W], F32R, tag=f"kiu_{e}")
            dma(ki0[:], kiT_d.ap()[e, 0:128])
            dma(ki1[:], kiT_d.ap()[e, 128:256])
            dma(kiu[:], kiT_d.ap()[e, 256:258])
            ki[e] = (ki0, ki1, kiu)
            ub = const.tile([128, W], F32R, tag=f"ubc_{e}")
            nc.gpsimd.partition_broadcast(ub[:], kiu[0:1, :])
            ubc[e] = ub
            uc = const.tile([128, 6], F32, tag=f"ucol_{e}")
            dma(uc[:], ucol_d.ap()[e])
            ucol_t[e] = uc

            att = apool.tile([128, 2, HA], F32R, tag=f"att{e}")
            for vt in range(2):
                ps = ps_b.tile([128, 512], F32, tag="psb")
                ch = [(ki0[:, 512 + vt * 128: 512 + (vt + 1) * 128], dsw_t[:, 0, :]),
                      (ki1[:, 512 + vt * 128: 512 + (vt + 1) * 128], dsw_t[:, 1, :])]
                if ds_bias:
                    ch.append((kiu[:, 512 + vt * 128: 512 + (vt + 1) * 128],
                               dsu_t[:, :]))
                mm(ps[:], ch)
                evac(att[:, vt, :], ps[:])
            attT = tpool.tile([128, 4, NV], F32R, tag=f"attT{e}")
            for t in range(4):
                ps = ps_b.tile([128, 512], F32, tag="psb")
                ch = [(dsw_t[:, 0, ts(t, 128)], ki0[:, 512:768]),
                      (dsw_t[:, 1, ts(t, 128)], ki1[:, 512:768])]
                if ds_bias:
                    ch.append((dsu_t[:, ts(t, 128)], kiu[:, 512:768]))
                mm(ps[:, 0:NV], ch)
                evac(attT[:, t, :], ps[:, 0:NV])
            state[e] = (att, attT)

        # ================== layers ==================
        for l in range(L):
            kw_t = wpool.tile([128, 2, HA], F32R, tag="kw")
            dma(kw_t[:], kw_d.ap()[l, 0:256].rearrange("(a p) n -> p a n", p=128))
            vw_t = wpool.tile([128, 2, HA], F32R, tag="vw")
            dma(vw_t[:], vw_d.ap()[l, 0:256].rearrange("(a p) n -> p a n", p=128))
            vwu_sb = wpool.tile([1, HA], F32R, tag="vwu_sb")
            dma(vwu_sb[:], vw_d.ap()[l, 256:257])
            vwu_b = wpool.tile([128, HA], F32R, tag="vwu_b")
            nc.gpsimd.partition_broadcast(vwu_b[:], vwu_sb[0:1, :])
            if kv_bias:
                vb_sb = wpool.tile([1, HA], F32R, tag="vb_sb")
                dma(vb_sb[:], vw_d.ap()[l, 257:258])
                vb_b = wpool.tile([128, HA], F32R, tag="vb_b")
                nc.gpsimd.partition_broadcast(vb_b[:], vb_sb[0:1, :])
            ffw1_t = fpool.tile([128, 4, M], F32R, tag="f1")
            dma(ffw1_t[:], f1_d.ap()[l].rearrange("(a p) n -> p a n", p=128))
            ffw2_t = fpool.tile([128, 4, M], F32R, tag="f2")
            dma(ffw2_t[:], f2_d.ap()[l].rearrange("(a p) n -> p a n", p=128))
            ffw3_t = fpool.tile([128, 4, HA], F32R, tag="f3")
            dma(ffw3_t[:], f3_d.ap()[l].rearrange("(a p) n -> p a n", p=128))

            for e in range(EPC):
                ki0, ki1, kiu = ki[e]
                att, attT = state[e]

                # ---- keysT [ha, w]; u-row folded via stt on evac ----
                keysT = kvpool.tile([128, 4, W], F32R, tag="keysT")
                for t in range(4):
                    for (wlo, wn) in ((0, 512), (512, 256)):
                        ps = ps_b.tile([128, 512], F32, tag="psb")
                        mm(ps[:, 0:wn],
                           [(kw_t[:, 0, ts(t, 128)], ki0[:, wlo:wlo + wn]),
                            (kw_t[:, 1, ts(t, 128)], ki1[:, wlo:wlo + wn])])
                        nc.vector.scalar_tensor_tensor(
                            keysT[:, t, wlo:wlo + wn],
                            ubc[e][:, wlo:wlo + wn],
                            kwu_t[:, l * 4 + t: l * 4 + t + 1],
                            ps[:, 0:wn],
                            op0=ALU.mult, op1=ALU.add)
                        if kv_bias:
                            nc.scalar.activation(
                                keysT[:, t, wlo:wlo + wn],
                                keysT[:, t, wlo:wlo + wn], AF.Identity,
                                bias=kbc_t[:, l * 4 + t: l * 4 + t + 1])

                # ---- vals [w, (h, a|1)]; ones col for softmax denom ----
                vals = kvpool.tile([128, 6, 8, 66], F32R, tag="vals")
                nc.gpsimd.tensor_copy(
                    vals[:, :, :, 64:66],
                    vones_t[:].rearrange("p (a b c) -> p a b c", a=6, b=8))
                for wt in range(6):
                    ps = ps_b.tile([128, 512], F32, tag="psb")
                    mm(ps[:], [(ki0[:, ts(wt, 128)], vw_t[:, 0, :]),
                               (ki1[:, ts(wt, 128)], vw_t[:, 1, :])])
                    nc.vector.scalar_tensor_tensor(
                        vals[:, wt, :, 0:64],
                        vwu_b[:].rearrange("p (h a) -> p h a", h=8),
                        ucol_t[e][:, wt:wt + 1],
                        ps[:].rearrange("p (h a) -> p h a", h=8),
                        op0=ALU.mult, op1=ALU.add)
                    if kv_bias:
                        nc.vector.tensor_add(
                            vals[:, wt, :, 0:64],
                            vals[:, wt, :, 0:64],
                            vb_b[:].rearrange("p (h a) -> p h a", h=8))

                # ---- attention ----
                att_res = apool.tile([128, 2, HA], F32R, tag=f"att{e}")
                for h in range(H):
                    t, base = h // 2, (h % 2) * 64
                    expT = epool.tile([128, 1536], F32R, tag="exp")
                    for pair in range(3):
                        ps = ps_sc.tile([128, 512], F32, tag="sc")
                        for k in range(2):
                            wt = 2 * pair + k
                            nc.tensor.matmul(
                                ps[:, ts(k, 256)],
                                keysT[base:base + 64, t, ts(wt, 128)],
                                attT[base:base + 64, t, :],
                                start=True, stop=True)
                        nc.scalar.activation(expT[:, ts(pair, 512)], ps[:],
                                             AF.Exp, scale=sc8_t[:, 0:1])
                    # masks on GpSimd (Pool): triangular blocks + dead block
                    nc.gpsimd.tensor_mul(expT[:, 1024:1152],
                                         expT[:, 1024:1152], maskm[:])
                    nc.gpsimd.tensor_mul(expT[:, 1408:1536],
                                         expT[:, 1408:1536], maskm[:])
                    nc.gpsimd.tensor_scalar_mul(expT[:, 1280:1408], expT[:, 1280:1408], 0.0)
                    ps_at = ps_a.tile([66, 256], F32, tag="a")
                    for wt in range(6):
                        nc.tensor.matmul(ps_at[:], vals[:, wt, h, :],
                                         expT[:, ts(wt, 256)],
                                         start=(wt == 0), stop=(wt == 5))
                    aT_s = spool.tile([66, 256], F32R, tag="aTs")
                    nc.scalar.copy(aT_s[:], ps_at[:])
                    ps_tr = ps_b.tile([128, 512], F32R, tag="psb")
                    rec = spool.tile([128, 2], F32, tag="rec")
                    for half in range(2):
                        nc.tensor.transpose(ps_tr[:, half * 66:half * 66 + 66],
                                            aT_s[:, ts(half, 128)],
                                            ident[0:66, 0:66])
                    for half in range(2):
                        nc.vector.reciprocal(rec[:, half:half + 1],
                                             ps_tr[:, half * 66 + 64:half * 66 + 65])
                    for half in range(2):
                        nc.vector.scalar_tensor_tensor(
                            att_res[:, half, ts(h, 64)],
                            ps_tr[:, half * 66:half * 66 + 64],
                            rec[:, half:half + 1],
                            att[:, half, ts(h, 64)],
                            op0=ALU.mult, op1=ALU.add)

                # ---- LN1 ----
                att1 = apool.tile([128, 2, HA], F32R, tag=f"att{e}")
                for vt in range(2):
                    ln_apply(att1[:, vt, :], att_res[:, vt, :], l, 1)
                att1T = tpool.tile([128, 4, NV], F32R, tag=f"attT{e}")
                for c in range(4):
                    ps_tr = ps_b.tile([128, 512], F32R, tag="psb")
                    for vt in range(2):
                        nc.tensor.transpose(ps_tr[:, ts(vt, 128)],
                                            att1[:, vt, ts(c, 128)], ident[:])
                    evac(att1T[:, c, :], ps_tr[:, 0:256])

                # ---- FF ----
                ff1T = ftpool.tile([128, 4, NV], F32R, tag="ffT")
                for mt in range(4):
                    ps = ps_b.tile([128, 512], F32, tag="psb")
                    mm(ps[:, 0:NV],
                       [(ffw1_t[:, c, ts(mt, 128)], att1T[:, c, :])
                        for c in range(4)])
                    nc.scalar.activation(
                        ff1T[:, mt, :], ps[:, 0:NV], AF.Relu,
                        bias=ffb_t[:, l * 4 + mt: l * 4 + mt + 1])
                ff2T = ftpool.tile([128, 4, NV], F32R, tag="ffT")
                for mt in range(4):
                    ps = ps_b.tile([128, 512], F32, tag="psb")
                    mm(ps[:, 0:NV],
                       [(ffw2_t[:, c, ts(mt, 128)], ff1T[:, c, :])
                        for c in range(4)])
                    # relu(x + bias) on DVE: (x add b) max 0
                    nc.vector.tensor_scalar(
                        ff2T[:, mt, :], ps[:, 0:NV],
                        ffb_t[:, (L + l) * 4 + mt: (L + l) * 4 + mt + 1], 0.0,
                        op0=ALU.add, op1=ALU.max)
                att2_res = apool.tile([128, 2, HA], F32R, tag=f"att{e}")
                for c in range(4):
                    ps3 = ps_b.tile([128, 512], F32, tag="psb")
                    mm(ps3[:, 0:NV],
                       [(ffw3_t[:, k, ts(c, 128)], ff2T[:, k, :])
                        for k in range(4)])
                    f3s = ftpool.tile([128, NV], F32R, tag="f3s")
                    nc.scalar.activation(
                        f3s[:], ps3[:, 0:NV], AF.Identity,
                        bias=ffb_t[:, (2 * L + l) * 4 + c: (2 * L + l) * 4 + c + 1])
                    ps_tr = ps_b.tile([128, 512], F32R, tag="psb")
                    for vt in range(2):
                        nc.tensor.transpose(ps_tr[:, ts(vt, 128)],
                                            f3s[:, ts(vt, 128)], ident[:])
                    for vt in range(2):
                        nc.vector.tensor_add(att2_res[:, vt, ts(c, 128)],
                                             ps_tr[:, ts(vt, 128)],
                                             att1[:, vt, ts(c, 128)])

                # ---- LN2 ----
                att2 = apool.tile([128, 2, HA], F32R, tag=f"att{e}")
                for vt in range(2):
                    ln_apply(att2[:, vt, :], att2_res[:, vt, :], l, 2)
                att2T = tpool.tile([128, 4, NV], F32R, tag=f"attT{e}")
                for c in range(4):
                    ps_tr = ps_b.tile([128, 512], F32R, tag="psb")
                    for vt in range(2):
                        nc.tensor.transpose(ps_tr[:, ts(vt, 128)],
                                            att2[:, vt, ts(c, 128)], ident[:])
                    evac(att2T[:, c, :], ps_tr[:, 0:256])
                state[e] = (att2, att2T)

        # ================== loss head ==================
        for e in range(EPC):
            att, attT = state[e]
            ps_l = ps_b.tile([128, 512], F32, tag="psb")
            mm(ps_l[:, 0:NV],
               [(dew_t[:, c, :], attT[:, c, :]) for c in range(4)])
            dpt = lpool.tile([128, 2 * NV], F32R, tag="dpt")
            nc.scalar.activation(dpt[:, 0:NV], ps_l[:, 0:NV], AF.Exp,
                                 bias=debcol[:, 0:1])
            nc.vector.scalar_tensor_tensor(
                dpt[:, NV:2 * NV], ps_l[:, 0:NV], debcol[:, 0:1],
                ohT_t[:, e, :], op0=ALU.add, op1=ALU.mult)
            ps2 = ps_b.tile([128, 512], F32, tag="psb")
            nc.tensor.matmul(ps2[0:1, :], onesc[:, 0:1], dpt[:],
                             start=True, stop=True)
            lse = lpool.tile([1, NV], F32, tag="lse")
            nc.scalar.activation(lse[:], ps2[0:1, 0:NV], AF.Ln)
            q = lpool.tile([1, NV], F32, tag="q")
            nc.vector.tensor_sub(q[:], ps2[0:1, NV:2 * NV], lse[:])
            nc.gpsimd.memset(q[0:1, 0:1], 0.0)
            tot = lpool.tile([1, 1], F32, tag="tot")
            nc.vector.tensor_reduce(tot[:], q[:], mybir.AxisListType.X, ALU.add)
            nc.scalar.activation(res_sb[0:1, e:e + 1], tot[0:1, 0:1],
                                 AF.Identity, scale=neg1_t[0:1, 0:1],
                                 bias=fbias_t[0:1, 0:1])
        dma(out_d.ap()[0:1, :], res_sb[:])

    nc.finalize()
    return nc


def _prep_inputs(inputs):
    hist_encoded = np.asarray(inputs["hist_encoded"], np.float32)
    hist_true_u = np.asarray(inputs["hist_true_u"], np.float32)
    pred_encoded = np.asarray(inputs["pred_encoded"], np.float32)
    pred_true_u = np.asarray(inputs["pred_true_u"], np.float32)
    key_w = np.asarray(inputs["key_w"], np.float32)
    key_b = np.asarray(inputs["key_b"], np.float32)
    val_w = np.asarray(inputs["val_w"], np.float32)
    val_b = np.asarray(inputs["val_b"], np.float32)
    ds_w = np.asarray(inputs["ds_w"], np.float32)
    ds_b = np.asarray(inputs["ds_b"], np.float32)
    ff_w1 = np.asarray(inputs["ff_w1"], np.float32)
    ff_b1 = np.asarray(inputs["ff_b1"], np.float32)
    ff_w2 = np.asarray(inputs["ff_w2"], np.float32)
    ff_b2 = np.asarray(inputs["ff_b2"], np.float32)
    ff_w3 = np.asarray(inputs["ff_w3"], np.float32)
    ff_b3 = np.asarray(inputs["ff_b3"], np.float32)
    de_w = np.asarray(inputs["de_w"], np.float32)
    de_b = np.asarray(inputs["de_b"], np.float32)
    ln1_g = np.asarray(inputs["ln1_g"], np.float32)
    ln1_b = np.asarray(inputs["ln1_b"], np.float32)
    ln2_g = np.asarray(inputs["ln2_g"], np.float32)
    ln2_b = np.asarray(inputs["ln2_b"], np.float32)

    # kiT per batch elem: [258, W]
    enc = np.concatenate([hist_encoded, pred_encoded], axis=1)  # [B, W, D]
    u = np.concatenate([hist_true_u, pred_true_u], axis=1)      # [B, W]
    kiT = np.empty((B, 258, W), np.float32)
    kiT[:, 0:256, :] = enc.transpose(0, 2, 1)
    kiT[:, 256, :] = u
    kiT[:, 257, :] = 1.0

    ucol = u.reshape(B, 6, 128).transpose(0, 2, 1).copy()  # [B, 128, 6]

    def pack_kv(wt, bt):  # [L,H,257,A],[L,H,A] -> [L,258,HA]
        p = np.empty((L, 258, HA), np.float32)
        p[:, 0:257, :] = wt.transpose(0, 2, 1, 3).reshape(L, 257, HA)
        p[:, 257, :] = bt.reshape(L, HA)
        return p

    kwp = pack_kv(key_w, key_b)
    vwp = pack_kv(val_w, val_b)

    # u-weight of keys as per-partition columns per ha-chunk: [128, L*4]
    kwucol = kwp[:, 256, :].reshape(L, 4, 128).transpose(2, 0, 1).reshape(128, L * 4).copy()
    kbcol = np.concatenate(
        [kwp[:, 257, :].reshape(L, 4, 128).transpose(2, 0, 1).reshape(128, L * 4),
         vwp[:, 257, :].reshape(L, 4, 128).transpose(2, 0, 1).reshape(128, L * 4)],
        axis=1)  # [128, 2*L*4] (keys cols first; vals bias handled separately)

    dswp = np.zeros((258, HA), np.float32)
    dswp[0:256] = ds_w
    dswp[257] = ds_b

    # ff bias columns [128, 3*L*4]: mi-major, then layer, then chunk
    ffbcol = np.empty((128, 3 * L * 4), np.float32)
    for mi, bt in enumerate((ff_b1, ff_b2, ff_b3)):
        for l in range(L):
            for t in range(4):
                ffbcol[:, (mi * L + l) * 4 + t] = bt[l, t * 128:(t + 1) * 128]

    rho = np.arange(128)[:, None]
    vv = np.arange(128)[None, :]
    maskmul = (vv > rho).astype(np.float32)  # 0 where masked (v <= r)

    tgt = np.clip(np.floor(pred_true_u * R).astype(np.int64), 0, R - 1)  # [B, NV]
    onehotT = np.zeros((B, R, NV), np.float32)
    bidx = np.arange(B)[:, None]
    vidx = np.arange(NV)[None, :]
    onehotT[bidx, tgt, vidx] = 1.0
    onehotT[:, :, 0] = 0.0  # exclude v=0

    ident = np.eye(128, dtype=np.float32)

    kv_bias = bool(np.any(key_b) or np.any(val_b))
    ds_bias = bool(np.any(ds_b))
    ln_affine = bool(np.any(ln1_g != 1.0) or np.any(ln1_b) or
                     np.any(ln2_g != 1.0) or np.any(ln2_b))
    lnp = np.stack([ln1_g, ln1_b, ln2_g, ln2_b], axis=1)  # [L,4,HA]

    shared = {
        "kwp": kwp, "vwp": vwp, "dswp": dswp,
        "ffw1": ff_w1, "ffw2": ff_w2, "ffw3": ff_w3,
        "ffbcol": ffbcol, "kwucol": kwucol,
        "dew": de_w, "debcol": de_b.reshape(R, 1),
        "maskmul": maskmul, "ident": ident,
        "onescol": np.ones((128, 1), np.float32),
        "vones": np.tile(np.array([1.0, 0.0], np.float32), 48).reshape(1, 96).repeat(128, 0),
    }
    if kv_bias:
        shared["kbcol"] = kbcol
    if ln_affine:
        shared["lnp"] = lnp
    in_maps = []
    for c in range(NCORES):
        m = dict(shared)
        m["kiT"] = kiT[c * EPC:(c + 1) * EPC]
        m["ucol"] = ucol[c * EPC:(c + 1) * EPC]
        m["onehotT"] = onehotT[c * EPC:(c + 1) * EPC]
        in_maps.append(m)
    return in_maps, (kv_bias, ds_bias, ln_affine)


def _get_nc(flags):
    if flags not in _BUILD_CACHE:
        _BUILD_CACHE[flags] = _build(*flags)
    return _BUILD_CACHE[flags]


def _run(inputs, trace=False):
    from concourse.bass_utils import run_bass_kernel_spmd
    in_maps, flags = _prep_inputs(inputs)
    nc = _get_nc(flags)
    res = run_bass_kernel_spmd(nc, in_maps, list(range(NCORES)), trace=trace)
    out = np.concatenate([res.results[c]["out"].reshape(EPC)
                          for c in range(NCORES)])
    return out.astype(np.float32), res


def kernel(**inputs) -> np.ndarray:
    out, _ = _run(inputs, trace=False)
    return out


# revision 15
# speedup vs baseline: 1.2362x; 1.0268x over previous
"""AttentionalCopula Trainium2 kernel.

Data-parallel over batch: 8 NeuronCores, 2 batch elements per core.
Perf structure:
  - weights DMA'd once per layer (shared by both batch elements),
    double-buffered for prefetch; ff weights in bf16 (halves DMA)
  - keys/vals double-buffered so the next (layer, elem) projection GEMMs
    overlap attention (keeps TensorE dense -> HAM clock stays at 2.4GHz)
  - attention operand path (keysT/vals/expT/attT/ffT) in bf16: frees SBUF,
    2x DVE, 1.0 cyc/row transposes; f32 residual stream + LN kept in fp32
  - QK scores in two psum tiles per head ([128,1024] spanning 2 banks);
    exp runs as 2 ACT instrs; AV accumulates into the spare bank
  - u-row contribution of key/val projections folded into the psum
    evacuation as a DVE scalar_tensor_tensor (drops the K=2 matmul chains)
  - LayerNorm 1/sigma = exp(-0.5*ln(var+eps)): keeps ACT on the single
    natural_log_exp table set (no ACT_TABLE_LOAD switches; Sqrt banned)
  - loss head restructured so all matmuls have moving free dim >= 256
    (fp32r matmuls with N<256 run at 1/4 rate)

Self-contained: hardcodes shapes from the problem spec.
"""
import math
import sys

import numpy as np

sys.path.insert(0, "/opt/trn_rl_repo")

import ml_dtypes  # noqa: E402
import concourse.bass as bass  # noqa: E402
import concourse.bacc as bacc  # noqa: E402
import concourse.tile as tile  # noqa: E402
import concourse.mybir as mybir  # noqa: E402
from contextlib import ExitStack  # noqa: E402

F32 = mybir.dt.float32
F32R = mybir.dt.float32r
BF16 = mybir.dt.bfloat16
AF = mybir.ActivationFunctionType
ALU = mybir.AluOpType
NPBF = ml_dtypes.bfloat16

B, D, NH, NS, NT = 16, 256, 512, 8, 32
NV = NS * NT
L, H, A = 4, 8, 64
HA = H * A
M = 512
R = 128
W = NH + NV
EPS = 1e-5
SCALE = A ** -0.5
NCORES = 8
EPC = B // NCORES  # elems per core

_BUILD_CACHE = {}


def ts(i, n):
    return slice(i * n, (i + 1) * n)


def _build(kv_bias, ds_bias, ln_affine):
    nc = bacc.Bacc(None, target_bir_lowering=False)

    def P(name, shape, out=False, dt=F32):
        return nc.declare_dram_parameter(name, shape, dt, isOutput=out)

    kiT_d = P("kiT", (EPC, 258, W), dt=F32R)
    ucol_d = P("ucol", (EPC, 128, 6))
    kw_d = P("kwp", (L, 258, HA), dt=F32R)
    vw_d = P("vwp", (L, 258, HA), dt=F32R)
    ds_d = P("dswp", (258, HA), dt=F32R)
    f1_d = P("ffw1", (L, 512, M), dt=BF16)
    f2_d = P("ffw2", (L, 512, M), dt=BF16)
    f3_d = P("ffw3", (L, 512, HA), dt=BF16)
    ffb_d = P("ffbcol", (128, 3 * L * 4))
    kwu_d = P("kwucol", (128, L * 4))
    dew_d = P("dew", (HA, R), dt=F32R)
    deb_d = P("debcol", (128, 1))
    mask_d = P("maskmul", (128, 128))
    qk3_d = P("qk3bits", (128, 3))
    ohT_d = P("onehotT", (EPC, 128, NV))
    id_d = P("ident", (128, 128), dt=F32R)
    onesc_d = P("onescol", (128, 1), dt=F32R)
    vones_d = P("vones", (128, 96), dt=F32R)
    if kv_bias:
        kbc_d = P("kbcol", (128, 2 * L * 4))
    if ln_affine:
        lnp_d = P("lnp", (L, 4, HA))
    out_d = P("out", (1, EPC), out=True)

    with tile.TileContext(nc) as tc, ExitStack() as ctx:
        const = ctx.enter_context(tc.tile_pool(name="const", bufs=1))
        wpool = ctx.enter_context(tc.tile_pool(name="wts", bufs=2))
        fpool = ctx.enter_context(tc.tile_pool(name="ffw", bufs=2))
        kvpool = ctx.enter_context(tc.tile_pool(name="keys", bufs=2))
        epool = ctx.enter_context(tc.tile_pool(name="exp", bufs=3))
        apool = ctx.enter_context(tc.tile_pool(name="att", bufs=3))
        tpool = ctx.enter_context(tc.tile_pool(name="attT", bufs=2))
        ftpool = ctx.enter_context(tc.tile_pool(name="ffT", bufs=2))
        spool = ctx.enter_context(tc.tile_pool(name="small", bufs=3))
        lpool = ctx.enter_context(tc.tile_pool(name="loss", bufs=2))
        ps_sc = ctx.enter_context(tc.tile_pool(name="ps_sc", bufs=3, space="PSUM"))
        ps_b = ctx.enter_context(tc.tile_pool(name="ps_b", bufs=2, space="PSUM"))

        dma = nc.sync.dma_start

        # ---- constants ----
        ident = const.tile([128, 128], F32R, tag="ident")
        dma(ident[:], id_d.ap())
        maskm = const.tile([128, 128], F32, tag="maskm")
        dma(maskm[:], mask_d.ap())
        ohT_t = const.tile([128, EPC, NV], F32, tag="ohT")
        for e in range(EPC):
            dma(ohT_t[:, e, :], ohT_d.ap()[e])
        onesc = const.tile([128, 1], F32R, tag="onesc")
        dma(onesc[:], onesc_d.ap())
        vones_t = const.tile([128, 96], F32R, tag="vones")
        dma(vones_t[:], vones_d.ap())
        dsw_t = const.tile([128, 2, HA], F32R, tag="dsw")
        dma(dsw_t[:], ds_d.ap()[0:256].rearrange("(a p) n -> p a n", p=128))
        if ds_bias:
            dsu_t = const.tile([2, HA], F32R, tag="dsu")
            dma(dsu_t[:], ds_d.ap()[256:258])
        dew_t = const.tile([128, 4, R], F32R, tag="dew")
        dma(dew_t[:], dew_d.ap().rearrange("(a p) n -> p a n", p=128))
        debcol = const.tile([128, 1], F32, tag="debcol")
        dma(debcol[:], deb_d.ap())
        ffb_t = const.tile([128, 3 * L * 4], F32, tag="ffb")
        dma(ffb_t[:], ffb_d.ap())
        kwu_t = const.tile([128, L * 4], F32, tag="kwu")
        dma(kwu_t[:], kwu_d.ap())
        if kv_bias:
            kbc_t = const.tile([128, 2 * L * 4], F32, tag="kbc")
            dma(kbc_t[:], kbc_d.ap())
        if ln_affine:
            lnp_t = const.tile([16, HA], F32, tag="lnp")
            for l in range(L):
                for j in range(4):
                    dma(lnp_t[l * 4 + j: l * 4 + j + 1, :], lnp_d.ap()[l, j: j + 1, :])
        qk3_t = const.tile([128, 3], F32, tag="qk3")
        dma(qk3_t[:], qk3_d.ap())
        c15_t = const.tile([128, 1], F32, tag="c15")
        nc.gpsimd.memset(c15_t[:], 1.5)
        res_sb = const.tile([1, EPC], F32, tag="res")
        eps_t = const.tile([128, 1], F32, tag="eps")
        nc.gpsimd.memset(eps_t[:], EPS)
        sc8_t = const.tile([128, 1], F32, tag="sc8")
        nc.gpsimd.memset(sc8_t[:], SCALE)
        neg1_t = const.tile([1, 1], F32, tag="neg1")
        nc.gpsimd.memset(neg1_t[:], -1.0)
        fbias_t = const.tile([1, 1], F32, tag="fbias")
        nc.gpsimd.memset(fbias_t[:], -(NV - 1) * math.log(R))

        evac_ctr = [0]

        def evac(out_ap, in_ap):
            # PSUM->SBUF copies, alternating DVE / ACT
            if evac_ctr[0] % 2 == 0:
                nc.vector.tensor_copy(out_ap, in_ap)
            else:
                nc.scalar.copy(out_ap, in_ap)
            evac_ctr[0] += 1

        def mm(ps_ap, chunks):
            n = len(chunks)
            for i, (lh, rh) in enumerate(chunks):
                nc.tensor.matmul(ps_ap, lh, rh,
                                 start=(i == 0), stop=(i == n - 1))

        def ln_apply(out_ap, in_ap, l, which):
            """LayerNorm along free dim (HA) of a [128, HA] slice.

            1/sigma computed as exp(-0.5*ln(var+eps)) so ACT stays on the
            natural_log_exp table set (Sqrt would force a table switch).
            """
            st6 = spool.tile([128, 6], F32, tag="st6")
            nc.vector.bn_stats(st6[:], in_ap)
            mv = spool.tile([128, 2], F32, tag="mv")
            nc.vector.bn_aggr(mv[:], st6[:])
            I32 = mybir.dt.int32
            vpe = spool.tile([128, 1], F32, tag="vpe")
            nc.vector.tensor_scalar_add(vpe[:], mv[:, 1:2], EPS)
            hv = spool.tile([128, 1], F32, tag="hv")
            nc.vector.tensor_scalar_mul(hv[:], vpe[:], 0.5)
            # Quake rsqrt: y0 = bits^-1(0x5f3759df - (bits(v) >> 1)),
            # then 2 Newton steps y' = y*(1.5 - h*y^2); all on DVE
            yq = spool.tile([128, 1], F32, tag="yq")
            nc.vector.tensor_scalar(yq[:].bitcast(I32), vpe[:].bitcast(I32),
                                    qk3_t[:, 0:1].bitcast(I32), None,
                                    op0=ALU.logical_shift_right)
            yn = spool.tile([128, 1], F32, tag="yn")
            nc.vector.tensor_scalar(yn[:].bitcast(I32), yq[:].bitcast(I32),
                                    qk3_t[:, 1:2].bitcast(I32), None,
                                    op0=ALU.bitwise_xor)
            rs = spool.tile([128, 1], F32, tag="rs")
            nc.vector.tensor_tensor(rs[:].bitcast(I32), yn[:].bitcast(I32),
                                    qk3_t[:, 2:3].bitcast(I32), op=ALU.add)
            for _ in range(2):
                t1 = spool.tile([128, 1], F32, tag="t1")
                nc.vector.tensor_mul(t1[:], rs[:], rs[:])
                mq = spool.tile([128, 1], F32, tag="mq")
                nc.vector.scalar_tensor_tensor(mq[:], t1[:], hv[:, 0:1],
                                               c15_t[:, 0:1],
                                               op0=ALU.mult, op1=ALU.subtract)
                rs2 = spool.tile([128, 1], F32, tag="rs")
                nc.vector.scalar_tensor_tensor(rs2[:], mq[:], -1.0, rs[:],
                                               op0=ALU.mult, op1=ALU.mult)
                rs = rs2
            nb = spool.tile([128, 1], F32, tag="nb")
            nc.vector.tensor_scalar(nb[:], mv[:, 0:1], rs[:, 0:1], -1.0,
                                    op0=ALU.mult, op1=ALU.mult)
            if not ln_affine:
                # (x * rs) + nb  on DVE
                nc.vector.tensor_scalar(out_ap, in_ap, rs[:, 0:1], nb[:, 0:1],
                                        op0=ALU.mult, op1=ALU.add)
            else:
                t0 = spool.tile([128, HA], F32, tag="lnt0")
                nc.vector.tensor_scalar(t0[:], in_ap, rs[:, 0:1], nb[:, 0:1],
                                        op0=ALU.mult, op1=ALU.add)
                gb = spool.tile([128, HA], F32, tag="lngb")
                gi = l * 4 + (0 if which == 1 else 2)
                nc.gpsimd.partition_broadcast(gb[:], lnp_t[gi: gi + 1, :])
                nc.vector.tensor_mul(t0[:], t0[:], gb[:])
                bi = gi + 1
                nc.gpsimd.partition_broadcast(gb[:], lnp_t[bi: bi + 1, :])
                nc.vector.tensor_add(out_ap, t0[:], gb[:])

        # ---- per-elem inputs + dimension-shift init ----
        ki = {}
        ubc = {}
        ucol_t = {}
        state = {}
        for e in range(EPC):
            ki0 = const.tile([128, W], F32R, tag=f"ki0_{e}")
            ki1 = const.tile([128, W], F32R, tag=f"ki1_{e}")
            kiu = const.tile([2, W], F32R, tag=f"kiu_{e}")
            dma(ki0[:], kiT_d.ap()[e, 0:128])
            dma(ki1[:], kiT_d.ap()[e, 128:256])
            dma(kiu[:], kiT_d.ap()[e, 256:258])
            ki[e] = (ki0, ki1, kiu)
            ub = const.tile([128, W], F32R, tag=f"ubc_{e}")
            nc.gpsimd.partition_broadcast(ub[:], kiu[0:1, :])
            ubc[e] = ub
            uc = const.tile([128, 6], F32, tag=f"ucol_{e}")
            dma(uc[:], ucol_d.ap()[e])
            ucol_t[e] = uc

            att = apool.tile([128, 2, HA], F32R, tag=f"att{e}")
            for vt in range(2):
                ps = ps_b.tile([128, 512], F32, tag="psb")
                ch = [(ki0[:, 512 + vt * 128: 512 + (vt + 1) * 128], dsw_t[:, 0, :]),
                      (ki1[:, 512 + vt * 128: 512 + (vt + 1) * 128], dsw_t[:, 1, :])]
                if ds_bias:
                    ch.append((kiu[:, 512 + vt * 128: 512 + (vt + 1) * 128],
                               dsu_t[:, :]))
                mm(ps[:], ch)
                evac(att[:, vt, :], ps[:])
            attT = tpool.tile([128, 4, NV], F32R, tag=f"attT{e}")
            for t in range(4):
                ps = ps_b.tile([128, 512], F32, tag="psb")
                ch = [(dsw_t[:, 0, ts(t, 128)], ki0[:, 512:768]),
                      (dsw_t[:, 1, ts(t, 128)], ki1[:, 512:768])]
                if ds_bias:
                    ch.append((dsu_t[:, ts(t, 128)], kiu[:, 512:768]))
                mm(ps[:, 0:NV], ch)
                evac(attT[:, t, :], ps[:, 0:NV])
            state[e] = (att, attT)

        # ================== layers ==================
        for l in range(L):
            kw_t = wpool.tile([128, 2, HA], F32R, tag="kw")
            dma(kw_t[:], kw_d.ap()[l, 0:256].rearrange("(a p) n -> p a n", p=128))
            vw_t = wpool.tile([128, 2, HA], F32R, tag="vw")
            dma(vw_t[:], vw_d.ap()[l, 0:256].rearrange("(a p) n -> p a n", p=128))
            vwu_sb = wpool.tile([1, HA], F32R, tag="vwu_sb")
            dma(vwu_sb[:], vw_d.ap()[l, 256:257])
            vwu_b = wpool.tile([128, HA], F32R, tag="vwu_b")
            nc.gpsimd.partition_broadcast(vwu_b[:], vwu_sb[0:1, :])
            if kv_bias:
                vb_sb = wpool.tile([1, HA], F32R, tag="vb_sb")
                dma(vb_sb[:], vw_d.ap()[l, 257:258])
                vb_b = wpool.tile([128, HA], F32R, tag="vb_b")
                nc.gpsimd.partition_broadcast(vb_b[:], vb_sb[0:1, :])
            ffw1_t = fpool.tile([128, 4, M], BF16, tag="f1")
            dma(ffw1_t[:], f1_d.ap()[l].rearrange("(a p) n -> p a n", p=128))
            ffw2_t = fpool.tile([128, 4, M], BF16, tag="f2")
            dma(ffw2_t[:], f2_d.ap()[l].rearrange("(a p) n -> p a n", p=128))
            ffw3_t = fpool.tile([128, 4, HA], BF16, tag="f3")
            dma(ffw3_t[:], f3_d.ap()[l].rearrange("(a p) n -> p a n", p=128))

            def kv_phase(e):
                ki0, ki1, kiu = ki[e]
                # keysT [ha, w]; u-row folded via stt on evac
                keysT = kvpool.tile([128, 4, W], F32R, tag="keysT")
                for t in range(4):
                    for (wlo, wn) in ((0, 512), (512, 256)):
                        ps = ps_b.tile([128, 512], F32, tag="psb")
                        mm(ps[:, 0:wn],
                           [(kw_t[:, 0, ts(t, 128)], ki0[:, wlo:wlo + wn]),
                            (kw_t[:, 1, ts(t, 128)], ki1[:, wlo:wlo + wn])])
                        nc.vector.scalar_tensor_tensor(
                            keysT[:, t, wlo:wlo + wn],
                            ubc[e][:, wlo:wlo + wn],
                            kwu_t[:, l * 4 + t: l * 4 + t + 1],
                            ps[:, 0:wn],
                            op0=ALU.mult, op1=ALU.add)
                        if kv_bias:
                            nc.scalar.activation(
                                keysT[:, t, wlo:wlo + wn],
                                keysT[:, t, wlo:wlo + wn], AF.Identity,
                                bias=kbc_t[:, l * 4 + t: l * 4 + t + 1])

                # vals [w, (h, a|1)]; ones col for softmax denom
                vals = kvpool.tile([128, 6, 8, 66], F32R, tag="vals")
                nc.gpsimd.tensor_copy(
                    vals[:, :, :, 64:66],
                    vones_t[:].rearrange("p (a b c) -> p a b c", a=6, b=8))
                for wt in range(6):
                    ps = ps_b.tile([128, 512], F32, tag="psb")
                    mm(ps[:], [(ki0[:, ts(wt, 128)], vw_t[:, 0, :]),
                               (ki1[:, ts(wt, 128)], vw_t[:, 1, :])])
                    nc.vector.scalar_tensor_tensor(
                        vals[:, wt, :, 0:64],
                        vwu_b[:].rearrange("p (h a) -> p h a", h=8),
                        ucol_t[e][:, wt:wt + 1],
                        ps[:].rearrange("p (h a) -> p h a", h=8),
                        op0=ALU.mult, op1=ALU.add)
                    if kv_bias:
                        nc.vector.tensor_add(
                            vals[:, wt, :, 0:64],
                            vals[:, wt, :, 0:64],
                            vb_b[:].rearrange("p (h a) -> p h a", h=8))
                return keysT, vals

            def attn_phase(e, keysT, vals):
                att, attT = state[e]
                att_res = apool.tile([128, 2, HA], F32R, tag=f"att{e}")

                def qk_part(h):
                    t, base = h // 2, (h % 2) * 64
                    expT = epool.tile([128, 1536], F32R, tag="exp")
                    # tileA: score chunks wt 0..3 (2 banks), one exp instr
                    psA = ps_sc.tile([128, 1024], F32, tag="sc")
                    for wt in range(4):
                        nc.tensor.matmul(
                            psA[:, ts(wt, 256)],
                            keysT[base:base + 64, t, ts(wt, 128)],
                            attT[base:base + 64, t, :],
                            start=True, stop=True)
                    nc.scalar.activation(expT[:, 0:1024], psA[:],
                                         AF.Exp, scale=sc8_t[:, 0:1])
                    # tileB: score chunks wt 4,5 in bank0; AV accumulates
                    # into bank1 (cols 512:768)
                    psB = ps_sc.tile([128, 1024], F32, tag="sc")
                    for wt in range(4, 6):
                        nc.tensor.matmul(
                            psB[:, ts(wt - 4, 256)],
                            keysT[base:base + 64, t, ts(wt, 128)],
                            attT[base:base + 64, t, :],
                            start=True, stop=True)
                    nc.scalar.activation(expT[:, 1024:1536], psB[:, 0:512],
                                         AF.Exp, scale=sc8_t[:, 0:1])
                    # triangular masks + dead block zero (DVE)
                    nc.vector.tensor_mul(expT[:, 1024:1152],
                                         expT[:, 1024:1152], maskm[:])
                    nc.vector.tensor_mul(expT[:, 1408:1536],
                                         expT[:, 1408:1536], maskm[:])
                    nc.vector.tensor_scalar_mul(expT[:, 1280:1408],
                                                expT[:, 1280:1408], 0.0)
                    return expT, psB

                def av_part(h, expT, psB):
                    for wt in range(6):
                        nc.tensor.matmul(psB[0:66, 512:768], vals[:, wt, h, :],
                                         expT[:, ts(wt, 256)],
                                         start=(wt == 0), stop=(wt == 5))
                    aT_s = spool.tile([66, 256], F32R, tag="aTs")
                    evac(aT_s[:], psB[0:66, 512:768])
                    ps_tr = ps_b.tile([128, 512], F32R, tag="psb")
                    rec = spool.tile([128, 2], F32, tag="rec")
                    for half in range(2):
                        nc.tensor.transpose(ps_tr[:, half * 66:half * 66 + 66],
                                            aT_s[:, ts(half, 128)],
                                            ident[0:66, 0:66])
                    for half in range(2):
                        nc.vector.reciprocal(rec[:, half:half + 1],
                                             ps_tr[:, half * 66 + 64:half * 66 + 65])
                    for half in range(2):
                        nc.vector.scalar_tensor_tensor(
                            att_res[:, half, ts(h, 64)],
                            ps_tr[:, half * 66:half * 66 + 64],
                            rec[:, half:half + 1],
                            att[:, half, ts(h, 64)],
                            op0=ALU.mult, op1=ALU.add)

                # software-pipelined: QK(h+1) is emitted before AV(h) so the
                # PE queue never waits on exp/mask of the current head
                pend = None
                for h in range(H):
                    cur = qk_part(h)
                    if pend is not None:
                        av_part(pend[0], *pend[1])
                    pend = (h, cur)
                av_part(pend[0], *pend[1])
                return att_res

            def lnff_phase(e, att_res):
                att, attT = state[e]
                # LN1
                att1 = apool.tile([128, 2, HA], F32R, tag=f"att{e}")
                for vt in range(2):
                    ln_apply(att1[:, vt, :], att_res[:, vt, :], l, 1)
                att1T = tpool.tile([128, 4, NV], BF16, tag=f"attT{e}")
                for c in range(4):
                    ps_tr = ps_b.tile([128, 512], F32R, tag="psb")
                    for vt in range(2):
                        nc.tensor.transpose(ps_tr[:, ts(vt, 128)],
                                            att1[:, vt, ts(c, 128)], ident[:])
                    evac(att1T[:, c, :], ps_tr[:, 0:256])

                # FF
                ff1T = ftpool.tile([128, 4, NV], BF16, tag="ffT")
                for mt in range(4):
                    ps = ps_b.tile([128, 512], F32, tag="psb")
                    mm(ps[:, 0:NV],
                       [(ffw1_t[:, c, ts(mt, 128)], att1T[:, c, :])
                        for c in range(4)])
                    nc.scalar.activation(
                        ff1T[:, mt, :], ps[:, 0:NV], AF.Relu,
                        bias=ffb_t[:, l * 4 + mt: l * 4 + mt + 1])
                ff2T = ftpool.tile([128, 4, NV], BF16, tag="ffT")
                for mt in range(4):
                    ps = ps_b.tile([128, 512], F32, tag="psb")
                    mm(ps[:, 0:NV],
                       [(ffw2_t[:, c, ts(mt, 128)], ff1T[:, c, :])
                        for c in range(4)])
                    nc.scalar.activation(
                        ff2T[:, mt, :], ps[:, 0:NV], AF.Relu,
                        bias=ffb_t[:, (L + l) * 4 + mt: (L + l) * 4 + mt + 1])
                att2_res = apool.tile([128, 2, HA], F32R, tag=f"att{e}")
                for c in range(4):
                    ps3 = ps_b.tile([128, 512], F32, tag="psb")
                    mm(ps3[:, 0:NV],
                       [(ffw3_t[:, k, ts(c, 128)], ff2T[:, k, :])
                        for k in range(4)])
                    f3s = ftpool.tile([128, NV], F32R, tag="f3s")
                    nc.scalar.activation(
                        f3s[:], ps3[:, 0:NV], AF.Identity,
                        bias=ffb_t[:, (2 * L + l) * 4 + c: (2 * L + l) * 4 + c + 1])
                    ps_tr = ps_b.tile([128, 512], F32R, tag="psb")
                    for vt in range(2):
                        nc.tensor.transpose(ps_tr[:, ts(vt, 128)],
                                            f3s[:, ts(vt, 128)], ident[:])
                    for vt in range(2):
                        nc.vector.tensor_add(att2_res[:, vt, ts(c, 128)],
                                             ps_tr[:, ts(vt, 128)],
                                             att1[:, vt, ts(c, 128)])

                # LN2
                att2 = apool.tile([128, 2, HA], F32R, tag=f"att{e}")
                for vt in range(2):
                    ln_apply(att2[:, vt, :], att2_res[:, vt, :], l, 2)
                att2T = tpool.tile([128, 4, NV], F32R, tag=f"attT{e}")
                for c in range(4):
                    ps_tr = ps_b.tile([128, 512], F32R, tag="psb")
                    for vt in range(2):
                        nc.tensor.transpose(ps_tr[:, ts(vt, 128)],
                                            att2[:, vt, ts(c, 128)], ident[:])
                    evac(att2T[:, c, :], ps_tr[:, 0:256])
                state[e] = (att2, att2T)

            # phase-interleaved emission: attention(e1) sits between
            # attention(e0) and ff(e0) in the PE queue, so LN/evac latency
            # of one elem is hidden behind the other's matmuls
            kv0 = kv_phase(0)
            kv1 = kv_phase(1)
            ar0 = attn_phase(0, *kv0)
            ar1 = attn_phase(1, *kv1)
            lnff_phase(0, ar0)
            lnff_phase(1, ar1)

        # ================== loss head ==================
        # both elems' Exp emitted before both Lns: one ACT table switch
        ps2s = {}
        for e in range(EPC):
            att, attT = state[e]
            ps_l = ps_b.tile([128, 512], F32, tag="psb")
            mm(ps_l[:, 0:NV],
               [(dew_t[:, c, :], attT[:, c, :]) for c in range(4)])
            dpt = lpool.tile([128, 2 * NV], F32R, tag="dpt")
            nc.scalar.activation(dpt[:, 0:NV], ps_l[:, 0:NV], AF.Exp,
                                 bias=debcol[:, 0:1])
            nc.vector.scalar_tensor_tensor(
                dpt[:, NV:2 * NV], ps_l[:, 0:NV], debcol[:, 0:1],
                ohT_t[:, e, :], op0=ALU.add, op1=ALU.mult)
            ps2 = ps_sc.tile([128, 1024], F32, tag="sc")
            nc.tensor.matmul(ps2[0:1, 0:512], onesc[:, 0:1], dpt[:],
                             start=True, stop=True)
            ps2s[e] = ps2
        for e in range(EPC):
            ps2 = ps2s[e]
            lse = lpool.tile([1, NV], F32, tag="lse")
            nc.scalar.activation(lse[:], ps2[0:1, 0:NV], AF.Ln)
            q = lpool.tile([1, NV], F32, tag="q")
            nc.vector.tensor_sub(q[:], ps2[0:1, NV:2 * NV], lse[:])
            nc.gpsimd.memset(q[0:1, 0:1], 0.0)
            tot = lpool.tile([1, 1], F32, tag="tot")
            nc.vector.tensor_reduce(tot[:], q[:], mybir.AxisListType.X, ALU.add)
            nc.scalar.activation(res_sb[0:1, e:e + 1], tot[0:1, 0:1],
                                 AF.Identity, scale=neg1_t[0:1, 0:1],
                                 bias=fbias_t[0:1, 0:1])
        dma(out_d.ap()[0:1, :], res_sb[:])

    nc.finalize()
    return nc


def _prep_inputs(inputs):
    hist_encoded = np.asarray(inputs["hist_encoded"], np.float32)
    hist_true_u = np.asarray(inputs["hist_true_u"], np.float32)
    pred_encoded = np.asarray(inputs["pred_encoded"], np.float32)
    pred_true_u = np.asarray(inputs["pred_true_u"], np.float32)
    key_w = np.asarray(inputs["key_w"], np.float32)
    key_b = np.asarray(inputs["key_b"], np.float32)
    val_w = np.asarray(inputs["val_w"], np.float32)
    val_b = np.asarray(inputs["val_b"], np.float32)
    ds_w = np.asarray(inputs["ds_w"], np.float32)
    ds_b = np.asarray(inputs["ds_b"], np.float32)
    ff_w1 = np.asarray(inputs["ff_w1"], np.float32)
    ff_b1 = np.asarray(inputs["ff_b1"], np.float32)
    ff_w2 = np.asarray(inputs["ff_w2"], np.float32)
    ff_b2 = np.asarray(inputs["ff_b2"], np.float32)
    ff_w3 = np.asarray(inputs["ff_w3"], np.float32)
    ff_b3 = np.asarray(inputs["ff_b3"], np.float32)
    de_w = np.asarray(inputs["de_w"], np.float32)
    de_b = np.asarray(inputs["de_b"], np.float32)
    ln1_g = np.asarray(inputs["ln1_g"], np.float32)
    ln1_b = np.asarray(inputs["ln1_b"], np.float32)
    ln2_g = np.asarray(inputs["ln2_g"], np.float32)
    ln2_b = np.asarray(inputs["ln2_b"], np.float32)

    # kiT per batch elem: [258, W]
    enc = np.concatenate([hist_encoded, pred_encoded], axis=1)  # [B, W, D]
    u = np.concatenate([hist_true_u, pred_true_u], axis=1)      # [B, W]
    kiT = np.empty((B, 258, W), np.float32)
    kiT[:, 0:256, :] = enc.transpose(0, 2, 1)
    kiT[:, 256, :] = u
    kiT[:, 257, :] = 1.0

    ucol = u.reshape(B, 6, 128).transpose(0, 2, 1).copy()  # [B, 128, 6]

    def pack_kv(wt, bt):  # [L,H,257,A],[L,H,A] -> [L,258,HA]
        p = np.empty((L, 258, HA), np.float32)
        p[:, 0:257, :] = wt.transpose(0, 2, 1, 3).reshape(L, 257, HA)
        p[:, 257, :] = bt.reshape(L, HA)
        return p

    kwp = pack_kv(key_w, key_b)
    vwp = pack_kv(val_w, val_b)

    # u-weight of keys as per-partition columns per ha-chunk: [128, L*4]
    kwucol = kwp[:, 256, :].reshape(L, 4, 128).transpose(2, 0, 1).reshape(128, L * 4).copy()
    kbcol = np.concatenate(
        [kwp[:, 257, :].reshape(L, 4, 128).transpose(2, 0, 1).reshape(128, L * 4),
         vwp[:, 257, :].reshape(L, 4, 128).transpose(2, 0, 1).reshape(128, L * 4)],
        axis=1)  # [128, 2*L*4] (keys cols used; vals bias via broadcast)

    dswp = np.zeros((258, HA), np.float32)
    dswp[0:256] = ds_w
    dswp[257] = ds_b

    # ff bias columns [128, 3*L*4]: mi-major, then layer, then chunk
    ffbcol = np.empty((128, 3 * L * 4), np.float32)
    for mi, bt in enumerate((ff_b1, ff_b2, ff_b3)):
        for l in range(L):
            for t in range(4):
                ffbcol[:, (mi * L + l) * 4 + t] = bt[l, t * 128:(t + 1) * 128]

    rho = np.arange(128)[:, None]
    vv = np.arange(128)[None, :]
    maskmul = (vv > rho).astype(np.float32)  # 0 where masked (v <= r)

    tgt = np.clip(np.floor(pred_true_u * R).astype(np.int64), 0, R - 1)  # [B, NV]
    onehotT = np.zeros((B, R, NV), np.float32)
    bidx = np.arange(B)[:, None]
    vidx = np.arange(NV)[None, :]
    onehotT[bidx, tgt, vidx] = 1.0
    onehotT[:, :, 0] = 0.0  # exclude v=0

    ident = np.eye(128, dtype=np.float32)

    qk3 = np.empty((128, 3), np.int32)
    qk3[:, 0] = 1                    # shift amount
    qk3[:, 1] = -1                   # xor all-ones
    qk3[:, 2] = 0x5F3759E0           # magic + 1
    qk3 = qk3.view(np.float32)

    kv_bias = bool(np.any(key_b) or np.any(val_b))
    ds_bias = bool(np.any(ds_b))
    ln_affine = bool(np.any(ln1_g != 1.0) or np.any(ln1_b) or
                     np.any(ln2_g != 1.0) or np.any(ln2_b))
    lnp = np.stack([ln1_g, ln1_b, ln2_g, ln2_b], axis=1)  # [L,4,HA]

    shared = {
        "kwp": kwp, "vwp": vwp, "dswp": dswp,
        "ffw1": ff_w1.astype(NPBF), "ffw2": ff_w2.astype(NPBF),
        "ffw3": ff_w3.astype(NPBF),
        "ffbcol": ffbcol, "kwucol": kwucol,
        "dew": de_w, "debcol": de_b.reshape(R, 1),
        "maskmul": maskmul, "ident": ident, "qk3bits": qk3,
        "onescol": np.ones((128, 1), np.float32),
        "vones": np.tile(np.array([1.0, 0.0], np.float32), 48).reshape(1, 96).repeat(128, 0),
    }
    if kv_bias:
        shared["kbcol"] = kbcol
    if ln_affine:
        shared["lnp"] = lnp
    in_maps = []
    for c in range(NCORES):
        m = dict(shared)
        m["kiT"] = kiT[c * EPC:(c + 1) * EPC]
        m["ucol"] = ucol[c * EPC:(c + 1) * EPC]
        m["onehotT"] = onehotT[c * EPC:(c + 1) * EPC]
        in_maps.append(m)
    return in_maps, (kv_bias, ds_bias, ln_affine)


def _get_nc(flags):
    if flags not in _BUILD_CACHE:
        _BUILD_CACHE[flags] = _build(*flags)
    return _BUILD_CACHE[flags]


def _run(inputs, trace=False):
    from concourse.bass_utils import run_bass_kernel_spmd
    in_maps, flags = _prep_inputs(inputs)
    nc = _get_nc(flags)
    res = run_bass_kernel_spmd(nc, in_maps, list(range(NCORES)), trace=trace)
    out = np.concatenate([res.results[c]["out"].reshape(EPC)
                          for c in range(NCORES)])
    return out.astype(np.float32), res


def kernel(**inputs) -> np.ndarray:
    out, _ = _run(inputs, trace=False)
    return out


# revision 18
# speedup vs baseline: 1.6758x; 1.3557x over previous
"""AttentionalCopula Trainium2 kernel.

Data-parallel over batch: 8 NeuronCores, 2 batch elements per core.
Perf structure:
  - weights DMA'd once per layer (shared by both batch elements),
    double-buffered for prefetch; ff weights in bf16 (halves DMA)
  - keys/vals double-buffered so the next (layer, elem) projection GEMMs
    overlap attention (keeps TensorE dense -> HAM clock stays at 2.4GHz)
  - attention operand path (keysT/vals/expT/attT/ffT) in bf16: frees SBUF,
    2x DVE, 1.0 cyc/row transposes; f32 residual stream + LN kept in fp32
  - QK scores in two psum tiles per head ([128,1024] spanning 2 banks);
    exp runs as 2 ACT instrs; AV accumulates into the spare bank
  - u-row contribution of key/val projections folded into the psum
    evacuation as a DVE scalar_tensor_tensor (drops the K=2 matmul chains)
  - LayerNorm 1/sigma = exp(-0.5*ln(var+eps)): keeps ACT on the single
    natural_log_exp table set (no ACT_TABLE_LOAD switches; Sqrt banned)
  - loss head restructured so all matmuls have moving free dim >= 256
    (fp32r matmuls with N<256 run at 1/4 rate)

Self-contained: hardcodes shapes from the problem spec.
"""
import math
import sys

import numpy as np

sys.path.insert(0, "/opt/trn_rl_repo")

import ml_dtypes  # noqa: E402
import concourse.bass as bass  # noqa: E402
import concourse.bacc as bacc  # noqa: E402
import concourse.tile as tile  # noqa: E402
import concourse.mybir as mybir  # noqa: E402
from contextlib import ExitStack  # noqa: E402

F32 = mybir.dt.float32
F32R = mybir.dt.float32r
BF16 = mybir.dt.bfloat16
AF = mybir.ActivationFunctionType
ALU = mybir.AluOpType
NPBF = ml_dtypes.bfloat16

B, D, NH, NS, NT = 16, 256, 512, 8, 32
NV = NS * NT
L, H, A = 4, 8, 64
HA = H * A
M = 512
R = 128
W = NH + NV
EPS = 1e-5
SCALE = A ** -0.5
NCORES = 8
EPC = B // NCORES  # elems per core

_BUILD_CACHE = {}


def ts(i, n):
    return slice(i * n, (i + 1) * n)


def _build(kv_bias, ds_bias, ln_affine):
    nc = bacc.Bacc(None, target_bir_lowering=False)

    def P(name, shape, out=False, dt=F32):
        return nc.declare_dram_parameter(name, shape, dt, isOutput=out)

    kiT_d = P("kiT", (EPC, 258, W), dt=F32R)
    ucol_d = P("ucol", (EPC, 128, 6))
    kw_d = P("kwp", (L, 258, HA), dt=F32R)
    vw_d = P("vwp", (L, 258, HA), dt=F32R)
    ds_d = P("dswp", (258, HA), dt=F32R)
    f1_d = P("ffw1", (L, 512, M), dt=BF16)
    f2_d = P("ffw2", (L, 512, M), dt=BF16)
    f3_d = P("ffw3", (L, 512, HA), dt=BF16)
    ffb_d = P("ffbcol", (128, 3 * L * 4))
    kwu_d = P("kwucol", (128, L * 4))
    dew_d = P("dew", (HA, R), dt=F32R)
    deb_d = P("debcol", (128, 1))
    mask_d = P("maskmul", (128, 128))
    qk3_d = P("qk3bits", (128, 4))
    ohT_d = P("onehotT", (EPC, 128, NV))
    id_d = P("ident", (128, 128), dt=F32R)
    onesc_d = P("onescol", (128, 1), dt=F32R)
    vones_d = P("vones", (128, 96), dt=F32R)
    if kv_bias:
        kbc_d = P("kbcol", (128, 2 * L * 4))
    if ln_affine:
        lnp_d = P("lnp", (L, 4, HA))
    out_d = P("out", (1, EPC), out=True)

    with tile.TileContext(nc) as tc, ExitStack() as ctx:
        const = ctx.enter_context(tc.tile_pool(name="const", bufs=1))
        wpool = ctx.enter_context(tc.tile_pool(name="wts", bufs=2))
        fpool = ctx.enter_context(tc.tile_pool(name="ffw", bufs=2))
        kvpool = ctx.enter_context(tc.tile_pool(name="keys", bufs=2))
        epool = ctx.enter_context(tc.tile_pool(name="exp", bufs=3))
        apool = ctx.enter_context(tc.tile_pool(name="att", bufs=3))
        tpool = ctx.enter_context(tc.tile_pool(name="attT", bufs=2))
        ftpool = ctx.enter_context(tc.tile_pool(name="ffT", bufs=2))
        spool = ctx.enter_context(tc.tile_pool(name="small", bufs=3))
        lpool = ctx.enter_context(tc.tile_pool(name="loss", bufs=2))
        ps_sc = ctx.enter_context(tc.tile_pool(name="ps_sc", bufs=3, space="PSUM"))
        ps_b = ctx.enter_context(tc.tile_pool(name="ps_b", bufs=2, space="PSUM"))

        dma = nc.sync.dma_start

        # ---- constants ----
        ident = const.tile([128, 128], F32R, tag="ident")
        dma(ident[:], id_d.ap())
        maskm = const.tile([128, 128], F32, tag="maskm")
        dma(maskm[:], mask_d.ap())
        ohT_t = const.tile([128, EPC, NV], F32, tag="ohT")
        for e in range(EPC):
            dma(ohT_t[:, e, :], ohT_d.ap()[e])
        onesc = const.tile([128, 1], F32R, tag="onesc")
        dma(onesc[:], onesc_d.ap())
        vones_t = const.tile([128, 96], F32R, tag="vones")
        dma(vones_t[:], vones_d.ap())
        dsw_t = const.tile([128, 2, HA], F32R, tag="dsw")
        dma(dsw_t[:], ds_d.ap()[0:256].rearrange("(a p) n -> p a n", p=128))
        if ds_bias:
            dsu_t = const.tile([2, HA], F32R, tag="dsu")
            dma(dsu_t[:], ds_d.ap()[256:258])
        dew_t = const.tile([128, 4, R], F32R, tag="dew")
        dma(dew_t[:], dew_d.ap().rearrange("(a p) n -> p a n", p=128))
        debcol = const.tile([128, 1], F32, tag="debcol")
        dma(debcol[:], deb_d.ap())
        ffb_t = const.tile([128, 3 * L * 4], F32, tag="ffb")
        dma(ffb_t[:], ffb_d.ap())
        kwu_t = const.tile([128, L * 4], F32, tag="kwu")
        dma(kwu_t[:], kwu_d.ap())
        if kv_bias:
            kbc_t = const.tile([128, 2 * L * 4], F32, tag="kbc")
            dma(kbc_t[:], kbc_d.ap())
        if ln_affine:
            lnp_t = const.tile([16, HA], F32, tag="lnp")
            for l in range(L):
                for j in range(4):
                    dma(lnp_t[l * 4 + j: l * 4 + j + 1, :], lnp_d.ap()[l, j: j + 1, :])
        qk3_t = const.tile([128, 4], F32, tag="qk3")
        dma(qk3_t[:], qk3_d.ap())
        res_sb = const.tile([1, EPC], F32, tag="res")
        eps_t = const.tile([128, 1], F32, tag="eps")
        nc.gpsimd.memset(eps_t[:], EPS)
        sc8_t = const.tile([128, 1], F32, tag="sc8")
        nc.gpsimd.memset(sc8_t[:], SCALE)
        neg1_t = const.tile([1, 1], F32, tag="neg1")
        nc.gpsimd.memset(neg1_t[:], -1.0)
        fbias_t = const.tile([1, 1], F32, tag="fbias")
        nc.gpsimd.memset(fbias_t[:], -(NV - 1) * math.log(R))

        evac_ctr = [0]

        def evac(out_ap, in_ap):
            # PSUM->SBUF copies, alternating DVE / ACT
            if evac_ctr[0] % 2 == 0:
                nc.vector.tensor_copy(out_ap, in_ap)
            else:
                nc.scalar.copy(out_ap, in_ap)
            evac_ctr[0] += 1

        def mm(ps_ap, chunks):
            n = len(chunks)
            for i, (lh, rh) in enumerate(chunks):
                nc.tensor.matmul(ps_ap, lh, rh,
                                 start=(i == 0), stop=(i == n - 1))

        def ln_pair(out_t, in_t, l, which):
            """LayerNorm along HA for both vt halves of a [128, 2, HA] tile.

            1/sigma via Quake rsqrt (bit trick + 2 Newton steps) entirely on
            DVE: keeps ACT free and on a single table set (Sqrt/Ln banned
            from the hot loop). Both halves share one [128, 2] chain.
            """
            I32 = mybir.dt.int32
            mvv = spool.tile([128, 2, 2], F32, tag="mvv")
            for vt in range(2):
                st6 = spool.tile([128, 6], F32, tag="st6")
                nc.vector.bn_stats(st6[:], in_t[:, vt, :])
                nc.vector.bn_aggr(mvv[:, vt, :], st6[:])
            vpe = spool.tile([128, 2], F32, tag="vpe")
            nc.vector.tensor_scalar_add(vpe[:], mvv[:, :, 1:2], EPS)
            hv = spool.tile([128, 2], F32, tag="hv")
            nc.vector.tensor_scalar_mul(hv[:], vpe[:], 0.5)
            yq = spool.tile([128, 2], F32, tag="yq")
            nc.vector.tensor_scalar(yq[:].bitcast(I32), vpe[:].bitcast(I32),
                                    qk3_t[:, 0:1].bitcast(I32), None,
                                    op0=ALU.logical_shift_right)
            yn = spool.tile([128, 2], F32, tag="yn")
            nc.vector.tensor_scalar(yn[:].bitcast(I32), yq[:].bitcast(I32),
                                    qk3_t[:, 1:2].bitcast(I32), None,
                                    op0=ALU.bitwise_xor)
            rs = spool.tile([128, 2], F32, tag="rs")
            nc.vector.tensor_tensor(rs[:].bitcast(I32), yn[:].bitcast(I32),
                                    qk3_t[:, 2:4].bitcast(I32), op=ALU.add)
            for _ in range(2):
                t1 = spool.tile([128, 2], F32, tag="t1")
                nc.vector.tensor_mul(t1[:], rs[:], rs[:])
                mq = spool.tile([128, 2], F32, tag="mq")
                nc.vector.tensor_mul(mq[:], t1[:], hv[:])
                uq = spool.tile([128, 2], F32, tag="uq")
                nc.vector.tensor_scalar(uq[:], mq[:], -1.0, 1.5,
                                        op0=ALU.mult, op1=ALU.add)
                rs2 = spool.tile([128, 2], F32, tag="rs")
                nc.vector.tensor_mul(rs2[:], uq[:], rs[:])
                rs = rs2
            nb = spool.tile([128, 2], F32, tag="nb")
            nc.vector.scalar_tensor_tensor(nb[:], mvv[:, :, 0:1], -1.0, rs[:],
                                           op0=ALU.mult, op1=ALU.mult)
            for vt in range(2):
                if not ln_affine:
                    nc.vector.tensor_scalar(out_t[:, vt, :], in_t[:, vt, :],
                                            rs[:, vt:vt + 1], nb[:, vt:vt + 1],
                                            op0=ALU.mult, op1=ALU.add)
                else:
                    t0 = spool.tile([128, HA], F32, tag="lnt0")
                    nc.vector.tensor_scalar(t0[:], in_t[:, vt, :],
                                            rs[:, vt:vt + 1], nb[:, vt:vt + 1],
                                            op0=ALU.mult, op1=ALU.add)
                    gb = spool.tile([128, HA], F32, tag="lngb")
                    gi = l * 4 + (0 if which == 1 else 2)
                    nc.gpsimd.partition_broadcast(gb[:], lnp_t[gi: gi + 1, :])
                    nc.vector.tensor_mul(t0[:], t0[:], gb[:])
                    bi = gi + 1
                    nc.gpsimd.partition_broadcast(gb[:], lnp_t[bi: bi + 1, :])
                    nc.vector.tensor_add(out_t[:, vt, :], t0[:], gb[:])

        # ---- per-elem inputs + dimension-shift init ----
        ki = {}
        ubc = {}
        ucol_t = {}
        state = {}
        for e in range(EPC):
            ki0 = const.tile([128, W], F32R, tag=f"ki0_{e}")
            ki1 = const.tile([128, W], F32R, tag=f"ki1_{e}")
            kiu = const.tile([2, W], F32R, tag=f"kiu_{e}")
            dma(ki0[:], kiT_d.ap()[e, 0:128])
            dma(ki1[:], kiT_d.ap()[e, 128:256])
            dma(kiu[:], kiT_d.ap()[e, 256:258])
            ki[e] = (ki0, ki1, kiu)
            ub = const.tile([128, W], F32R, tag=f"ubc_{e}")
            nc.gpsimd.partition_broadcast(ub[:], kiu[0:1, :])
            ubc[e] = ub
            uc = const.tile([128, 6], F32, tag=f"ucol_{e}")
            dma(uc[:], ucol_d.ap()[e])
            ucol_t[e] = uc

            att = apool.tile([128, 2, HA], F32R, tag=f"att{e}")
            for vt in range(2):
                ps = ps_b.tile([128, 512], F32, tag="psb")
                ch = [(ki0[:, 512 + vt * 128: 512 + (vt + 1) * 128], dsw_t[:, 0, :]),
                      (ki1[:, 512 + vt * 128: 512 + (vt + 1) * 128], dsw_t[:, 1, :])]
                if ds_bias:
                    ch.append((kiu[:, 512 + vt * 128: 512 + (vt + 1) * 128],
                               dsu_t[:, :]))
                mm(ps[:], ch)
                evac(att[:, vt, :], ps[:])
            attT = tpool.tile([128, 4, NV], F32R, tag=f"attT{e}")
            for t in range(4):
                ps = ps_b.tile([128, 512], F32, tag="psb")
                ch = [(dsw_t[:, 0, ts(t, 128)], ki0[:, 512:768]),
                      (dsw_t[:, 1, ts(t, 128)], ki1[:, 512:768])]
                if ds_bias:
                    ch.append((dsu_t[:, ts(t, 128)], kiu[:, 512:768]))
                mm(ps[:, 0:NV], ch)
                evac(attT[:, t, :], ps[:, 0:NV])
            state[e] = (att, attT)

        # ================== layers (software-pipelined) ==================
        def load_weights(l):
            w = {}
            w["kw"] = wpool.tile([128, 2, HA], F32R, tag="kw", name="w_kw")
            dma(w["kw"][:], kw_d.ap()[l, 0:256].rearrange("(a p) n -> p a n", p=128))
            w["vw"] = wpool.tile([128, 2, HA], F32R, tag="vw", name="w_vw")
            dma(w["vw"][:], vw_d.ap()[l, 0:256].rearrange("(a p) n -> p a n", p=128))
            vwu_sb = wpool.tile([1, HA], F32R, tag="vwu_sb")
            dma(vwu_sb[:], vw_d.ap()[l, 256:257])
            w["vwu"] = wpool.tile([128, HA], F32R, tag="vwu_b", name="w_vwu")
            nc.gpsimd.partition_broadcast(w["vwu"][:], vwu_sb[0:1, :])
            if kv_bias:
                vb_sb = wpool.tile([1, HA], F32R, tag="vb_sb")
                dma(vb_sb[:], vw_d.ap()[l, 257:258])
                w["vb"] = wpool.tile([128, HA], F32R, tag="vb_b", name="w_vb")
                nc.gpsimd.partition_broadcast(w["vb"][:], vb_sb[0:1, :])
            w["f1"] = fpool.tile([128, 4, M], BF16, tag="f1", name="w_f1")
            dma(w["f1"][:], f1_d.ap()[l].rearrange("(a p) n -> p a n", p=128))
            w["f2"] = fpool.tile([128, 4, M], BF16, tag="f2", name="w_f2")
            dma(w["f2"][:], f2_d.ap()[l].rearrange("(a p) n -> p a n", p=128))
            w["f3"] = fpool.tile([128, 4, HA], BF16, tag="f3", name="w_f3")
            dma(w["f3"][:], f3_d.ap()[l].rearrange("(a p) n -> p a n", p=128))
            return w

        def kv_phase(e, w, l):
            ki0, ki1, kiu = ki[e]
            # keysT [ha, w]; u-row folded via stt on evac; psum from the
            # 2-bank sc pool (two sub-chains per tile)
            keysT = kvpool.tile([128, 4, W], F32R, tag="keysT")
            for t in range(4):
                ps = ps_sc.tile([128, 1024], F32, tag="sc")
                for ci, (wlo, wn) in enumerate(((0, 512), (512, 256))):
                    pslice = ps[:, 512 * ci: 512 * ci + wn]
                    mm(pslice,
                       [(w["kw"][:, 0, ts(t, 128)], ki0[:, wlo:wlo + wn]),
                        (w["kw"][:, 1, ts(t, 128)], ki1[:, wlo:wlo + wn])])
                    nc.vector.scalar_tensor_tensor(
                        keysT[:, t, wlo:wlo + wn],
                        ubc[e][:, wlo:wlo + wn],
                        kwu_t[:, l * 4 + t: l * 4 + t + 1],
                        pslice,
                        op0=ALU.mult, op1=ALU.add)
                    if kv_bias:
                        nc.scalar.activation(
                            keysT[:, t, wlo:wlo + wn],
                            keysT[:, t, wlo:wlo + wn], AF.Identity,
                            bias=kbc_t[:, l * 4 + t: l * 4 + t + 1])

            # vals [w, (h, a|1)]; ones col for softmax denom
            vals = kvpool.tile([128, 6, 8, 66], F32R, tag="vals")
            nc.gpsimd.tensor_copy(
                vals[:, :, :, 64:66],
                vones_t[:].rearrange("p (a b c) -> p a b c", a=6, b=8))
            for wp in range(3):
                ps = ps_sc.tile([128, 1024], F32, tag="sc")
                for ci in range(2):
                    wt = 2 * wp + ci
                    pslice = ps[:, ts(ci, 512)]
                    mm(pslice, [(ki0[:, ts(wt, 128)], w["vw"][:, 0, :]),
                                (ki1[:, ts(wt, 128)], w["vw"][:, 1, :])])
                    nc.vector.scalar_tensor_tensor(
                        vals[:, wt, :, 0:64],
                        w["vwu"][:].rearrange("p (h a) -> p h a", h=8),
                        ucol_t[e][:, wt:wt + 1],
                        pslice.rearrange("p (h a) -> p h a", h=8),
                        op0=ALU.mult, op1=ALU.add)
                    if kv_bias:
                        nc.vector.tensor_add(
                            vals[:, wt, :, 0:64],
                            vals[:, wt, :, 0:64],
                            w["vb"][:].rearrange("p (h a) -> p h a", h=8))
            return keysT, vals

        def attn_phase(e, keysT, vals):
            att, attT = state[e]
            att_res = apool.tile([128, 2, HA], F32R, tag=f"att{e}")

            def qk_part(h):
                t, base = h // 2, (h % 2) * 64
                expT = epool.tile([128, 1536], F32R, tag="exp")
                # tileA: score chunks wt 0..3 (2 banks), one exp instr
                psA = ps_sc.tile([128, 1024], F32, tag="sc")
                for wt in range(4):
                    nc.tensor.matmul(
                        psA[:, ts(wt, 256)],
                        keysT[base:base + 64, t, ts(wt, 128)],
                        attT[base:base + 64, t, :],
                        start=True, stop=True)
                nc.scalar.activation(expT[:, 0:1024], psA[:],
                                     AF.Exp, scale=sc8_t[:, 0:1])
                # tileB: score chunks wt 4,5 in bank0; AV accumulates
                # into bank1 (cols 512:768)
                psB = ps_sc.tile([128, 1024], F32, tag="sc")
                for wt in range(4, 6):
                    nc.tensor.matmul(
                        psB[:, ts(wt - 4, 256)],
                        keysT[base:base + 64, t, ts(wt, 128)],
                        attT[base:base + 64, t, :],
                        start=True, stop=True)
                nc.scalar.activation(expT[:, 1024:1536], psB[:, 0:512],
                                     AF.Exp, scale=sc8_t[:, 0:1])
                # triangular masks + dead block zero (DVE)
                nc.vector.tensor_mul(expT[:, 1024:1152],
                                     expT[:, 1024:1152], maskm[:])
                nc.vector.tensor_mul(expT[:, 1408:1536],
                                     expT[:, 1408:1536], maskm[:])
                nc.vector.tensor_scalar_mul(expT[:, 1280:1408],
                                            expT[:, 1280:1408], 0.0)
                return expT, psB

            def av_part(h, expT, psB):
                for wt in range(6):
                    nc.tensor.matmul(psB[0:66, 512:768], vals[:, wt, h, :],
                                     expT[:, ts(wt, 256)],
                                     start=(wt == 0), stop=(wt == 5))
                aT_s = spool.tile([66, 256], F32R, tag="aTs")
                evac(aT_s[:], psB[0:66, 512:768])
                ps_tr = ps_b.tile([128, 512], F32R, tag="psb")
                rec = spool.tile([128, 2], F32, tag="rec")
                for half in range(2):
                    nc.tensor.transpose(ps_tr[:, half * 66:half * 66 + 66],
                                        aT_s[:, ts(half, 128)],
                                        ident[0:66, 0:66])
                for half in range(2):
                    nc.vector.reciprocal(rec[:, half:half + 1],
                                         ps_tr[:, half * 66 + 64:half * 66 + 65])
                for half in range(2):
                    nc.vector.scalar_tensor_tensor(
                        att_res[:, half, ts(h, 64)],
                        ps_tr[:, half * 66:half * 66 + 64],
                        rec[:, half:half + 1],
                        att[:, half, ts(h, 64)],
                        op0=ALU.mult, op1=ALU.add)

            # software-pipelined: QK(h+1) is emitted before AV(h) so the
            # PE queue never waits on exp/mask of the current head
            pend = None
            for h in range(H):
                cur = qk_part(h)
                if pend is not None:
                    av_part(pend[0], *pend[1])
                pend = (h, cur)
            av_part(pend[0], *pend[1])
            return att_res

        def ff_phase(e, w, l, att1):
            att1T = tpool.tile([128, 4, NV], BF16, tag=f"attT{e}")
            for c in range(4):
                ps_tr = ps_b.tile([128, 512], F32R, tag="psb")
                for vt in range(2):
                    nc.tensor.transpose(ps_tr[:, ts(vt, 128)],
                                        att1[:, vt, ts(c, 128)], ident[:])
                evac(att1T[:, c, :], ps_tr[:, 0:256])

            ff1T = ftpool.tile([128, 4, NV], BF16, tag="ffT")
            for mt in range(4):
                ps = ps_b.tile([128, 512], F32, tag="psb")
                mm(ps[:, 0:NV],
                   [(w["f1"][:, c, ts(mt, 128)], att1T[:, c, :])
                    for c in range(4)])
                nc.scalar.activation(
                    ff1T[:, mt, :], ps[:, 0:NV], AF.Relu,
                    bias=ffb_t[:, l * 4 + mt: l * 4 + mt + 1])
            ff2T = ftpool.tile([128, 4, NV], BF16, tag="ffT")
            for mt in range(4):
                ps = ps_b.tile([128, 512], F32, tag="psb")
                mm(ps[:, 0:NV],
                   [(w["f2"][:, c, ts(mt, 128)], ff1T[:, c, :])
                    for c in range(4)])
                nc.scalar.activation(
                    ff2T[:, mt, :], ps[:, 0:NV], AF.Relu,
                    bias=ffb_t[:, (L + l) * 4 + mt: (L + l) * 4 + mt + 1])
            att2_res = apool.tile([128, 2, HA], F32R, tag=f"att{e}")
            for c in range(4):
                ps3 = ps_b.tile([128, 512], F32, tag="psb")
                mm(ps3[:, 0:NV],
                   [(w["f3"][:, k, ts(c, 128)], ff2T[:, k, :])
                    for k in range(4)])
                f3s = ftpool.tile([128, NV], F32R, tag="f3s")
                nc.scalar.activation(
                    f3s[:], ps3[:, 0:NV], AF.Identity,
                    bias=ffb_t[:, (2 * L + l) * 4 + c: (2 * L + l) * 4 + c + 1])
                ps_tr = ps_b.tile([128, 512], F32R, tag="psb")
                for vt in range(2):
                    nc.tensor.transpose(ps_tr[:, ts(vt, 128)],
                                        f3s[:, ts(vt, 128)], ident[:])
                for vt in range(2):
                    nc.vector.tensor_add(att2_res[:, vt, ts(c, 128)],
                                         ps_tr[:, ts(vt, 128)],
                                         att1[:, vt, ts(c, 128)])
            return att2_res

        def attT_out(e, att2):
            att2T = tpool.tile([128, 4, NV], F32R, tag=f"attT{e}")
            for c in range(4):
                ps_tr = ps_b.tile([128, 512], F32R, tag="psb")
                for vt in range(2):
                    nc.tensor.transpose(ps_tr[:, ts(vt, 128)],
                                        att2[:, vt, ts(c, 128)], ident[:])
                evac(att2T[:, c, :], ps_tr[:, 0:256])
            return att2T

        wts = load_weights(0)
        kvs = {0: kv_phase(0, wts, 0), 1: kv_phase(1, wts, 0)}
        for l in range(L):
            nwts = load_weights(l + 1) if l + 1 < L else None
            ar0 = attn_phase(0, *kvs[0])
            ar1 = attn_phase(1, *kvs[1])
            # LN emissions are DVE-only; next layer's kv GEMMs are emitted
            # in between so the in-order PE queue always has ready work
            att1_0 = apool.tile([128, 2, HA], F32R, tag="att0")
            ln_pair(att1_0, ar0, l, 1)
            att1_1 = apool.tile([128, 2, HA], F32R, tag="att1")
            ln_pair(att1_1, ar1, l, 1)
            if nwts is not None:
                kvs[0] = kv_phase(0, nwts, l + 1)
            ar2_0 = ff_phase(0, wts, l, att1_0)
            att2_0 = apool.tile([128, 2, HA], F32R, tag="att0")
            ln_pair(att2_0, ar2_0, l, 2)
            if nwts is not None:
                kvs[1] = kv_phase(1, nwts, l + 1)
            ar2_1 = ff_phase(1, wts, l, att1_1)
            att2_1 = apool.tile([128, 2, HA], F32R, tag="att1")
            ln_pair(att2_1, ar2_1, l, 2)
            state[0] = (att2_0, attT_out(0, att2_0))
            state[1] = (att2_1, attT_out(1, att2_1))
            wts = nwts

        # ================== loss head ==================
        # both elems' Exp emitted before both Lns: one ACT table switch
        ps2s = {}
        for e in range(EPC):
            att, attT = state[e]
            ps_l = ps_b.tile([128, 512], F32, tag="psb")
            mm(ps_l[:, 0:NV],
               [(dew_t[:, c, :], attT[:, c, :]) for c in range(4)])
            dpt = lpool.tile([128, 2 * NV], F32R, tag="dpt")
            nc.scalar.activation(dpt[:, 0:NV], ps_l[:, 0:NV], AF.Exp,
                                 bias=debcol[:, 0:1])
            nc.vector.scalar_tensor_tensor(
                dpt[:, NV:2 * NV], ps_l[:, 0:NV], debcol[:, 0:1],
                ohT_t[:, e, :], op0=ALU.add, op1=ALU.mult)
            ps2 = ps_sc.tile([128, 1024], F32, tag="sc")
            nc.tensor.matmul(ps2[0:1, 0:512], onesc[:, 0:1], dpt[:],
                             start=True, stop=True)
            ps2s[e] = ps2
        for e in range(EPC):
            ps2 = ps2s[e]
            lse = lpool.tile([1, NV], F32, tag="lse")
            nc.scalar.activation(lse[:], ps2[0:1, 0:NV], AF.Ln)
            q = lpool.tile([1, NV], F32, tag="q")
            nc.vector.tensor_sub(q[:], ps2[0:1, NV:2 * NV], lse[:])
            nc.gpsimd.memset(q[0:1, 0:1], 0.0)
            tot = lpool.tile([1, 1], F32, tag="tot")
            nc.vector.tensor_reduce(tot[:], q[:], mybir.AxisListType.X, ALU.add)
            nc.scalar.activation(res_sb[0:1, e:e + 1], tot[0:1, 0:1],
                                 AF.Identity, scale=neg1_t[0:1, 0:1],
                                 bias=fbias_t[0:1, 0:1])
        dma(out_d.ap()[0:1, :], res_sb[:])

    nc.finalize()
    return nc


def _prep_inputs(inputs):
    hist_encoded = np.asarray(inputs["hist_encoded"], np.float32)
    hist_true_u = np.asarray(inputs["hist_true_u"], np.float32)
    pred_encoded = np.asarray(inputs["pred_encoded"], np.float32)
    pred_true_u = np.asarray(inputs["pred_true_u"], np.float32)
    key_w = np.asarray(inputs["key_w"], np.float32)
    key_b = np.asarray(inputs["key_b"], np.float32)
    val_w = np.asarray(inputs["val_w"], np.float32)
    val_b = np.asarray(inputs["val_b"], np.float32)
    ds_w = np.asarray(inputs["ds_w"], np.float32)
    ds_b = np.asarray(inputs["ds_b"], np.float32)
    ff_w1 = np.asarray(inputs["ff_w1"], np.float32)
    ff_b1 = np.asarray(inputs["ff_b1"], np.float32)
    ff_w2 = np.asarray(inputs["ff_w2"], np.float32)
    ff_b2 = np.asarray(inputs["ff_b2"], np.float32)
    ff_w3 = np.asarray(inputs["ff_w3"], np.float32)
    ff_b3 = np.asarray(inputs["ff_b3"], np.float32)
    de_w = np.asarray(inputs["de_w"], np.float32)
    de_b = np.asarray(inputs["de_b"], np.float32)
    ln1_g = np.asarray(inputs["ln1_g"], np.float32)
    ln1_b = np.asarray(inputs["ln1_b"], np.float32)
    ln2_g = np.asarray(inputs["ln2_g"], np.float32)
    ln2_b = np.asarray(inputs["ln2_b"], np.float32)

    # kiT per batch elem: [258, W]
    enc = np.concatenate([hist_encoded, pred_encoded], axis=1)  # [B, W, D]
    u = np.concatenate([hist_true_u, pred_true_u], axis=1)      # [B, W]
    kiT = np.empty((B, 258, W), np.float32)
    kiT[:, 0:256, :] = enc.transpose(0, 2, 1)
    kiT[:, 256, :] = u
    kiT[:, 257, :] = 1.0

    ucol = u.reshape(B, 6, 128).transpose(0, 2, 1).copy()  # [B, 128, 6]

    def pack_kv(wt, bt):  # [L,H,257,A],[L,H,A] -> [L,258,HA]
        p = np.empty((L, 258, HA), np.float32)
        p[:, 0:257, :] = wt.transpose(0, 2, 1, 3).reshape(L, 257, HA)
        p[:, 257, :] = bt.reshape(L, HA)
        return p

    kwp = pack_kv(key_w, key_b)
    vwp = pack_kv(val_w, val_b)

    # u-weight of keys as per-partition columns per ha-chunk: [128, L*4]
    kwucol = kwp[:, 256, :].reshape(L, 4, 128).transpose(2, 0, 1).reshape(128, L * 4).copy()
    kbcol = np.concatenate(
        [kwp[:, 257, :].reshape(L, 4, 128).transpose(2, 0, 1).reshape(128, L * 4),
         vwp[:, 257, :].reshape(L, 4, 128).transpose(2, 0, 1).reshape(128, L * 4)],
        axis=1)  # [128, 2*L*4] (keys cols used; vals bias via broadcast)

    dswp = np.zeros((258, HA), np.float32)
    dswp[0:256] = ds_w
    dswp[257] = ds_b

    # ff bias columns [128, 3*L*4]: mi-major, then layer, then chunk
    ffbcol = np.empty((128, 3 * L * 4), np.float32)
    for mi, bt in enumerate((ff_b1, ff_b2, ff_b3)):
        for l in range(L):
            for t in range(4):
                ffbcol[:, (mi * L + l) * 4 + t] = bt[l, t * 128:(t + 1) * 128]

    rho = np.arange(128)[:, None]
    vv = np.arange(128)[None, :]
    maskmul = (vv > rho).astype(np.float32)  # 0 where masked (v <= r)

    tgt = np.clip(np.floor(pred_true_u * R).astype(np.int64), 0, R - 1)  # [B, NV]
    onehotT = np.zeros((B, R, NV), np.float32)
    bidx = np.arange(B)[:, None]
    vidx = np.arange(NV)[None, :]
    onehotT[bidx, tgt, vidx] = 1.0
    onehotT[:, :, 0] = 0.0  # exclude v=0

    ident = np.eye(128, dtype=np.float32)

    qk3 = np.empty((128, 4), np.int32)
    qk3[:, 0] = 1                    # shift amount
    qk3[:, 1] = -1                   # xor all-ones
    qk3[:, 2] = 0x5F3759E0           # magic + 1
    qk3[:, 3] = 0x5F3759E0           # magic + 1 (second half)
    qk3 = qk3.view(np.float32)

    kv_bias = bool(np.any(key_b) or np.any(val_b))
    ds_bias = bool(np.any(ds_b))
    ln_affine = bool(np.any(ln1_g != 1.0) or np.any(ln1_b) or
                     np.any(ln2_g != 1.0) or np.any(ln2_b))
    lnp = np.stack([ln1_g, ln1_b, ln2_g, ln2_b], axis=1)  # [L,4,HA]

    shared = {
        "kwp": kwp, "vwp": vwp, "dswp": dswp,
        "ffw1": ff_w1.astype(NPBF), "ffw2": ff_w2.astype(NPBF),
        "ffw3": ff_w3.astype(NPBF),
        "ffbcol": ffbcol, "kwucol": kwucol,
        "dew": de_w, "debcol": de_b.reshape(R, 1),
        "maskmul": maskmul, "ident": ident, "qk3bits": qk3,
        "onescol": np.ones((128, 1), np.float32),
        "vones": np.tile(np.array([1.0, 0.0], np.float32), 48).reshape(1, 96).repeat(128, 0),
    }
    if kv_bias:
        shared["kbcol"] = kbcol
    if ln_affine:
        shared["lnp"] = lnp
    in_maps = []
    for c in range(NCORES):
        m = dict(shared)
        m["kiT"] = kiT[c * EPC:(c + 1) * EPC]
        m["ucol"] = ucol[c * EPC:(c + 1) * EPC]
        m["onehotT"] = onehotT[c * EPC:(c + 1) * EPC]
        in_maps.append(m)
    return in_maps, (kv_bias, ds_bias, ln_affine)


def _get_nc(flags):
    if flags not in _BUILD_CACHE:
        _BUILD_CACHE[flags] = _build(*flags)
    return _BUILD_CACHE[flags]


def _run(inputs, trace=False):
    from concourse.bass_utils import run_bass_kernel_spmd
    in_maps, flags = _prep_inputs(inputs)
    nc = _get_nc(flags)
    res = run_bass_kernel_spmd(nc, in_maps, list(range(NCORES)), trace=trace)
    out = np.concatenate([res.results[c]["out"].reshape(EPC)
                          for c in range(NCORES)])
    return out.astype(np.float32), res


def kernel(**inputs) -> np.ndarray:
    out, _ = _run(inputs, trace=False)
    return out
